# revision 11
# baseline (speedup 1.0000x reference)
"""DepthIoULoss kernel for Trainium2 (Bass/Tile), data-parallel over 8 cores.

Math (per row, S segments, 0-extended): with M = min(p, t), X = max(p, t)
elementwise and M[-1] = X[-1] = 0:
    inter_j = relu(M_j - X_{j-1});  union_j = X_j - M_{j-1};  iou = inter/union
Valid prefix: j <= stop_idx, where stop_idx = first index with t == 1.0.
row_iou = sum_valid iou_j / (stop_idx + 1);  loss = 1 - mean(row_iou).

Device plan per [128, 2048] tile pair (verified-native ops only):
  ACT    tq  = t * K                  (K = 1e31, Copy w/ scale)
  DVE    cmx = exclusive-cummax(tq)   (tensor_tensor_scan max, shifted APs)
  DVE    M   = min(p, t)
  GPSIMD X   = max(p, t)
  GPSIMD i0  = M[:,1:] - X[:,:-1]                  (pre-relu intersection)
  DVE    u0  = X[:,1:] - M[:,:-1]                  (union)
  DVE    u'  = max(cmx - 0.95K, u0), accum -> ia   (mask: invalid -> V=0.05K;
                                                    ia = n_invalid*V + O(1e3))
  ACT    ln  = Ln(u' + 1e-20)
  ACT    r   = Exp(-ln)   = 1/u'                   (invalid -> ~2e-30)
  DVE    junk= max(i0,0) * r, accum -> rowsum      (relu fused here)
Epilogue (once, [128, 8]): num_seg = S - round(ia / V) (magic-number round),
row_iou = rowsum / num_seg -> DMA out. Host: loss = 1 - sum(row_iou) / B.
"""

import numpy as np

B, S = 8192, 2048
NCORES = 8
ROWS_PER_CORE = B // NCORES  # 1024
TILES = ROWS_PER_CORE // 128  # 8

K_SCALE = np.float32(1.0e9)  # ACT Ln is only accurate to ~1e16; keep u' moderate
C_THRESH = np.float32(0.95) * K_SCALE  # f32 value of the threshold
V_INVALID = float(np.float32(K_SCALE - np.float32(C_THRESH)))  # u' on invalid lanes
MAGIC = 8388608.0  # 2**23: float add/sub rounds to nearest integer

_NC_CACHE = None

_RANGE_CLEAR_OPCODE = 176  # EVENT_SEMAPHORE_RANGE_CLEAR


def _legalize_waits(nc, maxw=1):
    """Make the Tile-generated module compatible with this walrus build.

    1. Drop tail EVENT_SEMAPHORE_RANGE_CLEAR InstISA ops (NRT re-initializes
       semaphore state per execution; this walrus rejects the encoding).
    2. Split instructions carrying more than `maxw` sync waits: excess waits
       move to carrier EventSemaphore nops inserted just before, same engine.
    """
    import concourse.mybir as mybir

    uid = [0]
    for fn in nc.m.functions:
        for blk in fn.blocks:
            lst = blk.instructions
            k = 0
            while k < len(lst):
                inst = lst[k]
                if (
                    type(inst).__name__ == "InstISA"
                    and getattr(inst, "isa_opcode", None) == _RANGE_CLEAR_OPCODE
                ):
                    si = inst.sync_info
                    if si is not None and (si.on_wait or si.on_update):
                        carrier = mybir.InstEventSemaphore(name=f"RCW-{uid[0]}")
                        uid[0] += 1
                        carrier.engine = inst.engine
                        carrier.sync_info = si
                        lst[k] = carrier
                        k += 1
                    else:
                        del lst[k]
                    continue
                si = inst.sync_info
                if si is not None and si.on_wait and len(si.on_wait) > maxw:
                    waits = list(si.on_wait)
                    extra, keep = waits[:-maxw], waits[-maxw:]
                    pos = k
                    for j in range(0, len(extra), maxw):
                        carrier = mybir.InstEventSemaphore(name=f"EVW-{uid[0]}")
                        uid[0] += 1
                        carrier.engine = inst.engine
                        carrier.sync_info = mybir.SyncInfo(
                            on_wait=extra[j : j + maxw], on_update=[]
                        )
                        lst.insert(pos, carrier)
                        pos += 1
                        k += 1
                    inst.sync_info = mybir.SyncInfo(
                        on_wait=keep, on_update=list(si.on_update)
                    )
                k += 1
    return nc


def _build_nc():
    import concourse.bass as bass
    import concourse.mybir as mybir
    from concourse.tile import TileContext

    f32 = mybir.dt.float32
    alu = mybir.AluOpType
    act = mybir.ActivationFunctionType

    nc = bass.Bass()
    p_d = nc.dram_tensor("predictions", [ROWS_PER_CORE, S], f32, kind="ExternalInput")
    t_d = nc.dram_tensor("targets", [ROWS_PER_CORE, S], f32, kind="ExternalInput")
    o_d = nc.dram_tensor("row_iou", [128, TILES], f32, kind="ExternalOutput")

    PAIRS = TILES // 2  # two row-tiles per 2 MB DMA (2 rows per partition)
    with TileContext(nc) as tc:
        with (
            tc.tile_pool(name="io", bufs=2) as iop,
            tc.tile_pool(name="geom", bufs=2) as gp,
            tc.tile_pool(name="work", bufs=2) as wp,
            tc.tile_pool(name="smp", bufs=1) as smp,
        ):
            # persistent small tiles
            lnbias = smp.tile([128, 1], f32, tag="lnbias")
            nc.vector.memset(lnbias[:], 1.0e-20)
            acc_sb = smp.tile([128, TILES], f32, tag="acc")
            rs_sb = smp.tile([128, TILES], f32, tag="rs")
            carr = smp.tile([128, TILES], f32, tag="carr")
            nc.vector.memset(carr[:], float(S) + MAGIC)
            out_sb = smp.tile([128, TILES], f32, tag="out")

            for pair in range(PAIRS):
                rows = slice(pair * 256, (pair + 1) * 256)
                # 2 MB loads: 2 DRAM rows land in each partition;
                # p over the SP HWDGE queue, t over the ACT HWDGE queue.
                p2 = iop.tile([128, 2, S], f32, tag="p")
                t2 = iop.tile([128, 2, S], f32, tag="t")
                src_p = p_d[rows, :].rearrange("(a b) c -> a b c", b=2)
                src_t = t_d[rows, :].rearrange("(a b) c -> a b c", b=2)
                nc.sync.dma_start(out=p2[:], in_=src_p)
                nc.sync.dma_start(out=t2[:], in_=src_t)

                for h in range(2):
                    i = pair * 2 + h
                    p = p2[:, h, :]
                    t = t2[:, h, :]

                    # tq = t * K  (ACT copy with scale)
                    tq = gp.tile([128, S], f32, tag="tq")
                    nc.scalar.activation(out=tq[:], in_=t, func=act.Copy,
                                         scale=float(K_SCALE))

                    # cmx = exclusive running max of tq (cmx[0] = 0)
                    cmx = gp.tile([128, S], f32, tag="cmx")
                    nc.vector.memset(cmx[:, 0:1], 0.0)
                    nc.vector.tensor_tensor_scan(
                        out=cmx[:, 1:S], data0=tq[:, 0 : S - 1],
                        data1=tq[:, 0 : S - 1],
                        initial=0.0, op0=alu.max, op1=alu.bypass,
                    )

                    # M/X with leading zero column (min/max: DVE only)
                    M = gp.tile([128, S + 1], f32, tag="M")
                    X = gp.tile([128, S + 1], f32, tag="X")
                    nc.vector.memset(M[:, 0:1], 0.0)
                    nc.gpsimd.memset(X[:, 0:1], 0.0)
                    nc.vector.tensor_tensor(
                        out=M[:, 1 : S + 1], in0=p, in1=t, op=alu.min
                    )
                    nc.vector.tensor_tensor(
                        out=X[:, 1 : S + 1], in0=p, in1=t, op=alu.max
                    )

                    # i0 = M[:,1:] - X[:,:-1]   (GPSIMD)
                    i0 = wp.tile([128, S], f32, tag="i0")
                    nc.gpsimd.tensor_tensor(
                        out=i0[:], in0=M[:, 1 : S + 1], in1=X[:, 0:S],
                        op=alu.subtract
                    )

                    # u0 = X[:,1:] - M[:,:-1]   (GPSIMD)
                    u0 = wp.tile([128, S], f32, tag="u0")
                    nc.gpsimd.tensor_tensor(
                        out=u0[:], in0=X[:, 1 : S + 1], in1=M[:, 0:S],
                        op=alu.subtract
                    )

                    # u' = max(cmx - C, u0); accum -> n_invalid * V (+ small)
                    um = wp.tile([128, S], f32, tag="um")
                    nc.vector.scalar_tensor_tensor(
                        out=um[:], in0=cmx[:], scalar=float(C_THRESH), in1=u0[:],
                        op0=alu.subtract, op1=alu.max,
                        accum_out=acc_sb[:, i : i + 1],
                    )

                    # r = 1/u' via exp(-ln(u' + 1e-20)) on ACT.
                    # lnu reuses cmx's tile, r reuses u0's (both dead by now).
                    nc.scalar.activation(out=cmx[:], in_=um[:], func=act.Ln,
                                         bias=lnbias[:])
                    nc.scalar.activation(out=u0[:], in_=cmx[:], func=act.Exp,
                                         scale=-1.0)

                    # rowsum = sum(relu(i0) * r); out reuses um's tile
                    nc.vector.scalar_tensor_tensor(
                        out=um[:], in0=i0[:], scalar=0.0, in1=u0[:],
                        op0=alu.max, op1=alu.mult,
                        accum_out=rs_sb[:, i : i + 1],
                    )

            # epilogue on [128, TILES]:
            # num_seg = (S + MAGIC - acc/V) - MAGIC  (rounds to integer)
            y = smp.tile([128, TILES], f32, tag="y")
            nc.vector.scalar_tensor_tensor(
                out=y[:], in0=acc_sb[:], scalar=-1.0 / V_INVALID, in1=carr[:],
                op0=alu.mult, op1=alu.add,
            )
            ns = smp.tile([128, TILES], f32, tag="ns")
            nc.vector.tensor_scalar(
                out=ns[:], in0=y[:], scalar1=MAGIC, scalar2=None, op0=alu.subtract
            )
            inv = smp.tile([128, TILES], f32, tag="inv")
            nc.vector.reciprocal(out=inv[:], in_=ns[:])
            nc.vector.tensor_tensor(
                out=out_sb[:], in0=rs_sb[:], in1=inv[:], op=alu.mult
            )
            nc.sync.dma_start(out=o_d[:, :], in_=out_sb[:])
    return _legalize_waits(nc)


def kernel(predictions: np.ndarray, targets: np.ndarray) -> np.ndarray:
    global _NC_CACHE
    from concourse.bass_utils import run_bass_kernel_spmd

    if _NC_CACHE is None:
        _NC_CACHE = _build_nc()
    nc = _NC_CACHE

    p = np.ascontiguousarray(predictions, dtype=np.float32)
    t = np.ascontiguousarray(targets, dtype=np.float32)
    in_maps = []
    for c in range(NCORES):
        sl = slice(c * ROWS_PER_CORE, (c + 1) * ROWS_PER_CORE)
        in_maps.append({"predictions": p[sl], "targets": t[sl]})
    res = run_bass_kernel_spmd(nc, in_maps, core_ids=list(range(NCORES)))
    total = 0.0
    for rmap in res.results:
        total += float(rmap["row_iou"].astype(np.float64).sum())
    return np.asarray(1.0 - total / B, dtype=np.float32)


# revision 12
# speedup vs baseline: 1.0379x; 1.0379x over previous
"""DepthIoULoss kernel for Trainium2 (Bass/Tile), data-parallel over 8 cores.

Math (per row, S segments; v[-1] treated as 0): with M = min(p, t) and
X = max(p, t) elementwise:
    inter_j = relu(M_j - X_{j-1});  union_j = X_j - M_{j-1};  iou = inter/union
Valid prefix: j <= stop_idx, where stop_idx = first index with t == 1.0.
row_iou = sum_valid iou_j / (stop_idx + 1);  loss = 1 - mean_rows(row_iou).

Device plan per [128, 2048] row-tile (only ops this walrus build accepts):
  ACT    tq  = t * K                      (K = 1e9, Copy w/ scale)
  DVE    cmx = exclusive-cummax(tq)       (tensor_tensor_scan max, shifted AP)
  DVE    M   = min(p, t);  X = max(p, t)  (min/max are DVE-only here)
  GPSIMD i0  = M[:,1:] - X[:,:-1]  (col 0 = M[:,0] via tiny ACT copy)
  GPSIMD u0  = X[:,1:] - M[:,:-1]  (col 0 = X[:,0])
  DVE    u'  = max(cmx - 0.95K, u0), accum -> ia    (invalid lanes -> V=0.05K;
                                                     ia = n_invalid*V + O(1e3))
  ACT    lnu = Ln(u');  r = Exp(-lnu) = 1/u'        (invalid -> 2e-8)
  DVE    junk= max(i0,0) * r, accum -> rowsum       (relu fused here)
Epilogue on [128, 8]: num_seg = S - round(ia / V) (2^23 magic rounding),
row_iou = rowsum / num_seg -> DMA out. Host: loss = 1 - sum(row_iou) / B.

The masked lanes contribute |inter|/V <= 2048 * 2e-8 ~ 4e-5 absolute to a
rowsum of O(1..30): negligible. num_seg recovery is exact (error << 0.5).
"""

import numpy as np

B, S = 8192, 2048
NCORES = 8
ROWS_PER_CORE = B // NCORES  # 1024
TILES = ROWS_PER_CORE // 128  # 8

K_SCALE = np.float32(1.0e9)  # ACT Ln accurate to ~1e16; keep u' moderate
C_THRESH = np.float32(0.95) * K_SCALE
V_INVALID = float(np.float32(K_SCALE - np.float32(C_THRESH)))  # invalid-lane u'
MAGIC = 8388608.0  # 2**23: float add/sub rounds to nearest integer

_NC_CACHE = None

_RANGE_CLEAR_OPCODE = 176  # EVENT_SEMAPHORE_RANGE_CLEAR


def _legalize_waits(nc, maxw=1):
    """Make the Tile-generated module compatible with this walrus build.

    1. Drop tail EVENT_SEMAPHORE_RANGE_CLEAR InstISA ops (NRT re-initializes
       semaphore state per execution; this walrus rejects the encoding).
    2. Split instructions carrying more than `maxw` sync waits: excess waits
       move to carrier EventSemaphore nops inserted just before, same engine.
    """
    import concourse.mybir as mybir

    uid = [0]
    for fn in nc.m.functions:
        for blk in fn.blocks:
            lst = blk.instructions
            k = 0
            while k < len(lst):
                inst = lst[k]
                if (
                    type(inst).__name__ == "InstISA"
                    and getattr(inst, "isa_opcode", None) == _RANGE_CLEAR_OPCODE
                ):
                    si = inst.sync_info
                    if si is not None and (si.on_wait or si.on_update):
                        carrier = mybir.InstEventSemaphore(name=f"RCW-{uid[0]}")
                        uid[0] += 1
                        carrier.engine = inst.engine
                        carrier.sync_info = si
                        lst[k] = carrier
                        k += 1
                    else:
                        del lst[k]
                    continue
                si = inst.sync_info
                if si is not None and si.on_wait and len(si.on_wait) > maxw:
                    waits = list(si.on_wait)
                    extra, keep = waits[:-maxw], waits[-maxw:]
                    pos = k
                    for j in range(0, len(extra), maxw):
                        carrier = mybir.InstEventSemaphore(name=f"EVW-{uid[0]}")
                        uid[0] += 1
                        carrier.engine = inst.engine
                        carrier.sync_info = mybir.SyncInfo(
                            on_wait=extra[j : j + maxw], on_update=[]
                        )
                        lst.insert(pos, carrier)
                        pos += 1
                        k += 1
                    inst.sync_info = mybir.SyncInfo(
                        on_wait=keep, on_update=list(si.on_update)
                    )
                k += 1
    return nc


def _build_nc():
    import concourse.bass as bass
    import concourse.mybir as mybir
    from concourse.tile import TileContext

    f32 = mybir.dt.float32
    alu = mybir.AluOpType
    act = mybir.ActivationFunctionType

    nc = bass.Bass()
    p_d = nc.dram_tensor("predictions", [ROWS_PER_CORE, S], f32, kind="ExternalInput")
    t_d = nc.dram_tensor("targets", [ROWS_PER_CORE, S], f32, kind="ExternalInput")
    o_d = nc.dram_tensor("row_iou", [128, TILES], f32, kind="ExternalOutput")

    with TileContext(nc) as tc:
        with (
            tc.tile_pool(name="io", bufs=3) as iop,
            tc.tile_pool(name="geom", bufs=2) as gp,
            tc.tile_pool(name="iu", bufs=2) as iup,
            tc.tile_pool(name="uch", bufs=2) as up,
            tc.tile_pool(name="smp", bufs=1) as smp,
        ):
            acc_sb = smp.tile([128, TILES], f32, tag="acc")
            rs_sb = smp.tile([128, TILES], f32, tag="rs")
            carr = smp.tile([128, TILES], f32, tag="carr")
            nc.vector.memset(carr[:], float(S) + MAGIC)
            w_sb = smp.tile([128, TILES], f32, tag="w")

            for i in range(TILES):
                rows = slice(i * 128, (i + 1) * 128)
                p = iop.tile([128, S], f32, tag="p")
                t = iop.tile([128, S], f32, tag="t")
                nc.sync.dma_start(out=t[:], in_=t_d[rows, :])
                nc.sync.dma_start(out=p[:], in_=p_d[rows, :])

                # tq = t * K
                tq = gp.tile([128, S], f32, tag="tq")
                nc.scalar.activation(
                    out=tq[:], in_=t[:], func=act.Copy, scale=float(K_SCALE)
                )

                # cmx = exclusive running max of tq (cmx[0] = 0)
                cmx = gp.tile([128, S], f32, tag="cmx")
                nc.vector.memset(cmx[:, 0:1], 0.0)
                nc.vector.tensor_tensor_scan(
                    out=cmx[:, 1:S],
                    data0=tq[:, 0 : S - 1],
                    data1=tq[:, 0 : S - 1],
                    initial=0.0,
                    op0=alu.max,
                    op1=alu.bypass,
                )

                M = gp.tile([128, S], f32, tag="M")
                X = gp.tile([128, S], f32, tag="X")
                nc.vector.tensor_tensor(out=M[:], in0=p[:], in1=t[:], op=alu.min)
                nc.vector.tensor_tensor(out=X[:], in0=p[:], in1=t[:], op=alu.max)

                # i0 = M_j - X_{j-1}; column 0 is M_0 - 0
                i0 = iup.tile([128, S], f32, tag="i0")
                nc.scalar.copy(i0[:, 0:1], M[:, 0:1])
                nc.gpsimd.tensor_tensor(
                    out=i0[:, 1:S], in0=M[:, 1:S], in1=X[:, 0 : S - 1],
                    op=alu.subtract,
                )
                # u0 = X_j - M_{j-1}; column 0 is X_0 - 0
                u0 = iup.tile([128, S], f32, tag="u0")
                nc.scalar.copy(u0[:, 0:1], X[:, 0:1])
                nc.gpsimd.tensor_tensor(
                    out=u0[:, 1:S], in0=X[:, 1:S], in1=M[:, 0 : S - 1],
                    op=alu.subtract,
                )

                # u' = max(cmx - C, u0); accum -> ~ n_invalid * V
                um = iup.tile([128, S], f32, tag="um")
                nc.vector.scalar_tensor_tensor(
                    out=um[:],
                    in0=cmx[:],
                    scalar=float(C_THRESH),
                    in1=u0[:],
                    op0=alu.subtract,
                    op1=alu.max,
                    accum_out=acc_sb[:, i : i + 1],
                )

                # r = 1/u' via exp(-ln(u')) on ACT
                lnu = up.tile([128, S], f32, tag="lnu")
                nc.scalar.activation(out=lnu[:], in_=um[:], func=act.Ln)
                r = up.tile([128, S], f32, tag="r")
                nc.scalar.activation(out=r[:], in_=lnu[:], func=act.Exp, scale=-1.0)

                # rowsum = sum(relu(i0) * r); scratch output reuses um's tile
                nc.vector.scalar_tensor_tensor(
                    out=um[:],
                    in0=i0[:],
                    scalar=0.0,
                    in1=r[:],
                    op0=alu.max,
                    op1=alu.mult,
                    accum_out=rs_sb[:, i : i + 1],
                )

            # epilogue: num_seg = (S + MAGIC - acc/V) - MAGIC; out = rs/num_seg
            nc.vector.scalar_tensor_tensor(
                out=w_sb[:], in0=acc_sb[:], scalar=-1.0 / V_INVALID, in1=carr[:],
                op0=alu.mult, op1=alu.add,
            )
            nc.vector.tensor_scalar(
                out=carr[:], in0=w_sb[:], scalar1=MAGIC, scalar2=None,
                op0=alu.subtract,
            )
            nc.vector.reciprocal(out=w_sb[:], in_=carr[:])
            nc.vector.tensor_tensor(
                out=carr[:], in0=rs_sb[:], in1=w_sb[:], op=alu.mult
            )
            nc.sync.dma_start(out=o_d[:, :], in_=carr[:])
    return _legalize_waits(nc)


def kernel(predictions: np.ndarray, targets: np.ndarray) -> np.ndarray:
    global _NC_CACHE
    from concourse.bass_utils import run_bass_kernel_spmd

    if _NC_CACHE is None:
        _NC_CACHE = _build_nc()
    nc = _NC_CACHE

    p = np.ascontiguousarray(predictions, dtype=np.float32)
    t = np.ascontiguousarray(targets, dtype=np.float32)
    in_maps = []
    for c in range(NCORES):
        sl = slice(c * ROWS_PER_CORE, (c + 1) * ROWS_PER_CORE)
        in_maps.append({"predictions": p[sl], "targets": t[sl]})
    res = run_bass_kernel_spmd(nc, in_maps, core_ids=list(range(NCORES)))
    total = 0.0
    for rmap in res.results:
        total += float(rmap["row_iou"].astype(np.float64).sum())
    return np.asarray(1.0 - total / B, dtype=np.float32)


# revision 13
# speedup vs baseline: 1.3941x; 1.3431x over previous
"""DepthIoULoss kernel for Trainium2 (Bass/Tile), data-parallel over 8 cores.

Math (per row, S segments; v[-1] treated as 0): with M = min(p, t) and
X = max(p, t) elementwise:
    inter_j = relu(M_j - X_{j-1});  union_j = X_j - M_{j-1};  iou = inter/union
Valid prefix: j <= stop_idx, where stop_idx = first index with t == 1.0.
row_iou = sum_valid iou_j / (stop_idx + 1);  loss = 1 - mean_rows(row_iou).

Device plan per [128, 2048] row-tile (only ops this walrus build accepts):
  ACT    tq  = t * K                      (K = 1e9, Copy w/ scale)
  DVE    cmx = exclusive-cummax(tq)       (tensor_tensor_scan max, shifted AP)
  DVE    M   = min(p, t);  X = max(p, t)  (min/max are DVE-only here;
                                           [128, S+1] tiles, zero column 0)
  GPSIMD i0  = M[:,1:] - X[:,:-1]
  GPSIMD u0  = X[:,1:] - M[:,:-1]
  DVE    u'  = max(cmx - 0.95K, u0), accum -> ia    (invalid lanes -> V=0.05K;
                                                     ia = n_invalid*V + O(1e3))
  ACT    lnu = Ln(u');  r = Exp(-lnu) = 1/u'        (invalid -> 2e-8)
  DVE    junk= max(i0,0) * r, accum -> rowsum       (relu fused here)
Epilogue on [128, 8]: num_seg = S - round(ia / V) (2^23 magic rounding),
row_iou = rowsum / num_seg -> DMA out. Host: loss = 1 - sum(row_iou) / B.

Manual software pipelining: engine queues run in EMISSION order, so the
um pass (which waits on GPSIMD's u0) is emitted one tile late and the
Ln/Exp/final passes two tiles late. This hides the Pool and ACT latency
behind the next tile's DVE work: sim went 140 us -> 104 us per core.

The masked lanes contribute |inter|/V <= 2048 * 2e-8 ~ 4e-5 absolute to a
rowsum of O(1..30): negligible. num_seg recovery is exact (error << 0.5).
"""

import numpy as np

B, S = 8192, 2048
NCORES = 8
ROWS_PER_CORE = B // NCORES  # 1024
TILES = ROWS_PER_CORE // 128  # 8

K_SCALE = np.float32(1.0e9)  # ACT Ln accurate to ~1e16; keep u' moderate
C_THRESH = np.float32(0.95) * K_SCALE
V_INVALID = float(np.float32(K_SCALE - np.float32(C_THRESH)))  # invalid-lane u'
MAGIC = 8388608.0  # 2**23: float add/sub rounds to nearest integer

UM_SKEW = 1  # um pass trails stage A by one tile
B_SKEW = 2  # ln/exp/final trail stage A by two tiles

_NC_CACHE = None

_RANGE_CLEAR_OPCODE = 176  # EVENT_SEMAPHORE_RANGE_CLEAR


def _legalize_waits(nc, maxw=1):
    """Make the Tile-generated module compatible with this walrus build.

    1. Drop tail EVENT_SEMAPHORE_RANGE_CLEAR InstISA ops (NRT re-initializes
       semaphore state per execution; this walrus rejects the encoding).
    2. Split instructions carrying more than `maxw` sync waits: excess waits
       move to carrier EventSemaphore nops inserted just before, same engine.
    """
    import concourse.mybir as mybir

    uid = [0]
    for fn in nc.m.functions:
        for blk in fn.blocks:
            lst = blk.instructions
            k = 0
            while k < len(lst):
                inst = lst[k]
                if (
                    type(inst).__name__ == "InstISA"
                    and getattr(inst, "isa_opcode", None) == _RANGE_CLEAR_OPCODE
                ):
                    si = inst.sync_info
                    if si is not None and (si.on_wait or si.on_update):
                        carrier = mybir.InstEventSemaphore(name=f"RCW-{uid[0]}")
                        uid[0] += 1
                        carrier.engine = inst.engine
                        carrier.sync_info = si
                        lst[k] = carrier
                        k += 1
                    else:
                        del lst[k]
                    continue
                si = inst.sync_info
                if si is not None and si.on_wait and len(si.on_wait) > maxw:
                    waits = list(si.on_wait)
                    extra, keep = waits[:-maxw], waits[-maxw:]
                    pos = k
                    for j in range(0, len(extra), maxw):
                        carrier = mybir.InstEventSemaphore(name=f"EVW-{uid[0]}")
                        uid[0] += 1
                        carrier.engine = inst.engine
                        carrier.sync_info = mybir.SyncInfo(
                            on_wait=extra[j : j + maxw], on_update=[]
                        )
                        lst.insert(pos, carrier)
                        pos += 1
                        k += 1
                    inst.sync_info = mybir.SyncInfo(
                        on_wait=keep, on_update=list(si.on_update)
                    )
                k += 1
    return nc


def _build_nc():
    import concourse.bass as bass
    import concourse.mybir as mybir
    from concourse.tile import TileContext

    f32 = mybir.dt.float32
    alu = mybir.AluOpType
    act = mybir.ActivationFunctionType

    nc = bass.Bass()
    p_d = nc.dram_tensor("predictions", [ROWS_PER_CORE, S], f32, kind="ExternalInput")
    t_d = nc.dram_tensor("targets", [ROWS_PER_CORE, S], f32, kind="ExternalInput")
    o_d = nc.dram_tensor("row_iou", [128, TILES], f32, kind="ExternalOutput")

    with TileContext(nc) as tc:
        with (
            tc.tile_pool(name="io", bufs=2) as iop,
            tc.tile_pool(name="geom", bufs=2) as gp,
            tc.tile_pool(name="cmxp", bufs=3) as cp,
            tc.tile_pool(name="i0p", bufs=2) as i0p,
            tc.tile_pool(name="u0p", bufs=2) as u0p,
            tc.tile_pool(name="ump", bufs=3) as ump,
            tc.tile_pool(name="uch", bufs=2) as up,
            tc.tile_pool(name="smp", bufs=1) as smp,
        ):
            acc_sb = smp.tile([128, TILES], f32, tag="acc")
            rs_sb = smp.tile([128, TILES], f32, tag="rs")
            carr = smp.tile([128, TILES], f32, tag="carr")
            nc.vector.memset(carr[:], float(S) + MAGIC)
            w_sb = smp.tile([128, TILES], f32, tag="w")

            st_a = {}
            st_u = {}

            def stage_a(i):
                rows = slice(i * 128, (i + 1) * 128)
                p = iop.tile([128, S], f32, tag="p")
                t = iop.tile([128, S], f32, tag="t")
                nc.sync.dma_start(out=t[:], in_=t_d[rows, :])
                nc.sync.dma_start(out=p[:], in_=p_d[rows, :])

                tq = gp.tile([128, S], f32, tag="tq")
                nc.scalar.activation(
                    out=tq[:], in_=t[:], func=act.Copy, scale=float(K_SCALE)
                )
                cmx = cp.tile([128, S], f32, tag="cmx")
                nc.vector.memset(cmx[:, 0:1], 0.0)
                nc.vector.tensor_tensor_scan(
                    out=cmx[:, 1:S],
                    data0=tq[:, 0 : S - 1],
                    data1=tq[:, 0 : S - 1],
                    initial=0.0,
                    op0=alu.max,
                    op1=alu.bypass,
                )

                M = gp.tile([128, S + 1], f32, tag="M")
                X = gp.tile([128, S + 1], f32, tag="X")
                nc.vector.memset(M[:, 0:1], 0.0)
                nc.gpsimd.memset(X[:, 0:1], 0.0)
                nc.vector.tensor_tensor(
                    out=M[:, 1 : S + 1], in0=p[:], in1=t[:], op=alu.min
                )
                nc.vector.tensor_tensor(
                    out=X[:, 1 : S + 1], in0=p[:], in1=t[:], op=alu.max
                )

                i0 = i0p.tile([128, S], f32, tag="i0")
                nc.gpsimd.tensor_tensor(
                    out=i0[:], in0=M[:, 1 : S + 1], in1=X[:, 0:S], op=alu.subtract
                )
                u0 = u0p.tile([128, S], f32, tag="u0")
                nc.gpsimd.tensor_tensor(
                    out=u0[:], in0=X[:, 1 : S + 1], in1=M[:, 0:S], op=alu.subtract
                )
                st_a[i] = (i0, u0, cmx)

            def stage_u(i):
                i0, u0, cmx = st_a.pop(i)
                um = ump.tile([128, S], f32, tag="um")
                nc.vector.scalar_tensor_tensor(
                    out=um[:],
                    in0=cmx[:],
                    scalar=float(C_THRESH),
                    in1=u0[:],
                    op0=alu.subtract,
                    op1=alu.max,
                    accum_out=acc_sb[:, i : i + 1],
                )
                st_u[i] = (i0, um)

            def stage_b(i):
                i0, um = st_u.pop(i)
                lnu = up.tile([128, S], f32, tag="lnu")
                nc.scalar.activation(out=lnu[:], in_=um[:], func=act.Ln)
                r = up.tile([128, S], f32, tag="r")
                nc.scalar.activation(out=r[:], in_=lnu[:], func=act.Exp, scale=-1.0)
                nc.vector.scalar_tensor_tensor(
                    out=um[:],
                    in0=i0[:],
                    scalar=0.0,
                    in1=r[:],
                    op0=alu.max,
                    op1=alu.mult,
                    accum_out=rs_sb[:, i : i + 1],
                )

            for i in range(TILES):
                stage_a(i)
                if i >= UM_SKEW:
                    stage_u(i - UM_SKEW)
                if i >= B_SKEW:
                    stage_b(i - B_SKEW)
            for i in range(TILES - UM_SKEW, TILES):
                stage_u(i)
            for i in range(TILES - B_SKEW, TILES):
                stage_b(i)

            # epilogue: num_seg = (S + MAGIC - acc/V) - MAGIC; out = rs/num_seg
            nc.vector.scalar_tensor_tensor(
                out=w_sb[:], in0=acc_sb[:], scalar=-1.0 / V_INVALID, in1=carr[:],
                op0=alu.mult, op1=alu.add,
            )
            nc.vector.tensor_scalar(
                out=carr[:], in0=w_sb[:], scalar1=MAGIC, scalar2=None,
                op0=alu.subtract,
            )
            nc.vector.reciprocal(out=w_sb[:], in_=carr[:])
            nc.vector.tensor_tensor(
                out=carr[:], in0=rs_sb[:], in1=w_sb[:], op=alu.mult
            )
            nc.sync.dma_start(out=o_d[:, :], in_=carr[:])
    return _legalize_waits(nc)


def kernel(predictions: np.ndarray, targets: np.ndarray) -> np.ndarray:
    global _NC_CACHE
    from concourse.bass_utils import run_bass_kernel_spmd

    if _NC_CACHE is None:
        _NC_CACHE = _build_nc()
    nc = _NC_CACHE

    p = np.ascontiguousarray(predictions, dtype=np.float32)
    t = np.ascontiguousarray(targets, dtype=np.float32)
    in_maps = []
    for c in range(NCORES):
        sl = slice(c * ROWS_PER_CORE, (c + 1) * ROWS_PER_CORE)
        in_maps.append({"predictions": p[sl], "targets": t[sl]})
    res = run_bass_kernel_spmd(nc, in_maps, core_ids=list(range(NCORES)))
    total = 0.0
    for rmap in res.results:
        total += float(rmap["row_iou"].astype(np.float64).sum())
    return np.asarray(1.0 - total / B, dtype=np.float32)


# revision 14
# speedup vs baseline: 1.4040x; 1.0071x over previous
"""DepthIoULoss kernel for Trainium2 (Bass/Tile), data-parallel over 8 cores.

Math (per row, S segments; v[-1] treated as 0): with M = min(p, t) and
X = max(p, t) elementwise:
    inter_j = relu(M_j - X_{j-1});  union_j = X_j - M_{j-1};  iou = inter/union
Valid prefix: j <= stop_idx, where stop_idx = first index with t == 1.0.
row_iou = sum_valid iou_j / (stop_idx + 1);  loss = 1 - mean_rows(row_iou).

Device plan per [128, 2048] row-tile (only ops this walrus build accepts):
  ACT    tq  = t * K                      (K = 1e9, Copy w/ scale)
  DVE    cmx = exclusive-cummax(tq)       (tensor_tensor_scan max, shifted AP)
  DVE    M   = min(p, t);  X = max(p, t)  (min/max are DVE-only here;
                                           [128, S+1] tiles, zero column 0)
  GPSIMD i0  = M[:,1:] - X[:,:-1]
  GPSIMD u0  = X[:,1:] - M[:,:-1]
  DVE    u'  = max(cmx - 0.95K, u0), accum -> ia    (invalid lanes -> V=0.05K;
                                                     ia = n_invalid*V + O(1e3))
  ACT    lnu = Ln(u');  r = Exp(-lnu) = 1/u'        (invalid -> 2e-8)
  DVE    junk= max(i0,0) * r, accum -> rowsum       (relu fused here)
Epilogue on [128, 8]: num_seg = S - round(ia / V) (2^23 magic rounding),
row_iou = rowsum / num_seg -> DMA out. Host: loss = 1 - sum(row_iou) / B.

Manual software pipelining: engine queues run in EMISSION order, so the
um pass (which waits on GPSIMD's u0) is emitted one tile late and the
Ln/Exp/final passes two tiles late. This hides the Pool and ACT latency
behind the next tile's DVE work: sim went 140 us -> 104 us per core.

The masked lanes contribute |inter|/V <= 2048 * 2e-8 ~ 4e-5 absolute to a
rowsum of O(1..30): negligible. num_seg recovery is exact (error << 0.5).
"""

import numpy as np

B, S = 8192, 2048
NCORES = 8
ROWS_PER_CORE = B // NCORES  # 1024
TILES = ROWS_PER_CORE // 128  # 8

K_SCALE = np.float32(1.0e9)  # ACT Ln accurate to ~1e16; keep u' moderate
C_THRESH = np.float32(0.95) * K_SCALE
V_INVALID = float(np.float32(K_SCALE - np.float32(C_THRESH)))  # invalid-lane u'
MAGIC = 8388608.0  # 2**23: float add/sub rounds to nearest integer

UM_SKEW = 1  # um pass trails stage A by one tile
B_SKEW = 2  # ln/exp/final trail stage A by two tiles

_NC_CACHE = None

_RANGE_CLEAR_OPCODE = 176  # EVENT_SEMAPHORE_RANGE_CLEAR


def _legalize_waits(nc, maxw=1):
    """Make the Tile-generated module compatible with this walrus build.

    1. Drop tail EVENT_SEMAPHORE_RANGE_CLEAR InstISA ops (NRT re-initializes
       semaphore state per execution; this walrus rejects the encoding).
    2. Split instructions carrying more than `maxw` sync waits: excess waits
       move to carrier EventSemaphore nops inserted just before, same engine.
    """
    import concourse.mybir as mybir

    uid = [0]
    for fn in nc.m.functions:
        for blk in fn.blocks:
            lst = blk.instructions
            k = 0
            while k < len(lst):
                inst = lst[k]
                if (
                    type(inst).__name__ == "InstISA"
                    and getattr(inst, "isa_opcode", None) == _RANGE_CLEAR_OPCODE
                ):
                    si = inst.sync_info
                    if si is not None and (si.on_wait or si.on_update):
                        carrier = mybir.InstEventSemaphore(name=f"RCW-{uid[0]}")
                        uid[0] += 1
                        carrier.engine = inst.engine
                        carrier.sync_info = si
                        lst[k] = carrier
                        k += 1
                    else:
                        del lst[k]
                    continue
                si = inst.sync_info
                if si is not None and si.on_wait and len(si.on_wait) > maxw:
                    waits = list(si.on_wait)
                    extra, keep = waits[:-maxw], waits[-maxw:]
                    pos = k
                    for j in range(0, len(extra), maxw):
                        carrier = mybir.InstEventSemaphore(name=f"EVW-{uid[0]}")
                        uid[0] += 1
                        carrier.engine = inst.engine
                        carrier.sync_info = mybir.SyncInfo(
                            on_wait=extra[j : j + maxw], on_update=[]
                        )
                        lst.insert(pos, carrier)
                        pos += 1
                        k += 1
                    inst.sync_info = mybir.SyncInfo(
                        on_wait=keep, on_update=list(si.on_update)
                    )
                k += 1
    return nc


def _build_nc():
    import concourse.bass as bass
    import concourse.mybir as mybir
    from concourse.tile import TileContext

    f32 = mybir.dt.float32
    alu = mybir.AluOpType
    act = mybir.ActivationFunctionType

    nc = bass.Bass()
    p_d = nc.dram_tensor("predictions", [ROWS_PER_CORE, S], f32, kind="ExternalInput")
    t_d = nc.dram_tensor("targets", [ROWS_PER_CORE, S], f32, kind="ExternalInput")
    o_d = nc.dram_tensor("row_iou", [128, TILES], f32, kind="ExternalOutput")

    with TileContext(nc) as tc:
        with (
            tc.tile_pool(name="io", bufs=2) as iop,
            tc.tile_pool(name="geom", bufs=2) as gp,
            tc.tile_pool(name="cmxp", bufs=3) as cp,
            tc.tile_pool(name="i0p", bufs=2) as i0p,
            tc.tile_pool(name="u0p", bufs=2) as u0p,
            tc.tile_pool(name="ump", bufs=3) as ump,
            tc.tile_pool(name="uch", bufs=2) as up,
            tc.tile_pool(name="smp", bufs=1) as smp,
        ):
            acc_sb = smp.tile([128, TILES], f32, tag="acc")
            rs_sb = smp.tile([128, TILES], f32, tag="rs")
            carr = smp.tile([128, TILES], f32, tag="carr")
            nc.vector.memset(carr[:], float(S) + MAGIC)
            w_sb = smp.tile([128, TILES], f32, tag="w")

            st_a = {}
            st_u = {}

            def stage_a(i):
                rows = slice(i * 128, (i + 1) * 128)
                p = iop.tile([128, S], f32, tag="p")
                t = iop.tile([128, S], f32, tag="t")
                nc.sync.dma_start(out=t[:], in_=t_d[rows, :])
                nc.sync.dma_start(out=p[:], in_=p_d[rows, :])

                tq = gp.tile([128, S], f32, tag="tq")
                nc.scalar.activation(
                    out=tq[:], in_=t[:], func=act.Copy, scale=float(K_SCALE)
                )
                cmx = cp.tile([128, S], f32, tag="cmx")
                nc.scalar.memzero(cmx[:, 0:1])
                nc.vector.tensor_tensor_scan(
                    out=cmx[:, 1:S],
                    data0=tq[:, 0 : S - 1],
                    data1=tq[:, 0 : S - 1],
                    initial=0.0,
                    op0=alu.max,
                    op1=alu.bypass,
                )

                M = gp.tile([128, S + 1], f32, tag="M")
                X = gp.tile([128, S + 1], f32, tag="X")
                nc.scalar.memzero(M[:, 0:1])
                nc.gpsimd.memset(X[:, 0:1], 0.0)
                nc.vector.tensor_tensor(
                    out=M[:, 1 : S + 1], in0=p[:], in1=t[:], op=alu.min
                )
                nc.vector.tensor_tensor(
                    out=X[:, 1 : S + 1], in0=p[:], in1=t[:], op=alu.max
                )

                i0 = i0p.tile([128, S], f32, tag="i0")
                nc.gpsimd.tensor_tensor(
                    out=i0[:], in0=M[:, 1 : S + 1], in1=X[:, 0:S], op=alu.subtract
                )
                u0 = u0p.tile([128, S], f32, tag="u0")
                nc.gpsimd.tensor_tensor(
                    out=u0[:], in0=X[:, 1 : S + 1], in1=M[:, 0:S], op=alu.subtract
                )
                st_a[i] = (i0, u0, cmx)

            def stage_u(i):
                i0, u0, cmx = st_a.pop(i)
                um = ump.tile([128, S], f32, tag="um")
                nc.vector.scalar_tensor_tensor(
                    out=um[:],
                    in0=cmx[:],
                    scalar=float(C_THRESH),
                    in1=u0[:],
                    op0=alu.subtract,
                    op1=alu.max,
                    accum_out=acc_sb[:, i : i + 1],
                )
                st_u[i] = (i0, um)

            def stage_b(i):
                i0, um = st_u.pop(i)
                lnu = up.tile([128, S], f32, tag="lnu")
                nc.scalar.activation(out=lnu[:], in_=um[:], func=act.Ln)
                r = up.tile([128, S], f32, tag="r")
                nc.scalar.activation(out=r[:], in_=lnu[:], func=act.Exp, scale=-1.0)
                nc.vector.scalar_tensor_tensor(
                    out=um[:],
                    in0=i0[:],
                    scalar=0.0,
                    in1=r[:],
                    op0=alu.max,
                    op1=alu.mult,
                    accum_out=rs_sb[:, i : i + 1],
                )

            for i in range(TILES):
                stage_a(i)
                if i >= UM_SKEW:
                    stage_u(i - UM_SKEW)
                if i >= B_SKEW:
                    stage_b(i - B_SKEW)
            for i in range(TILES - UM_SKEW, TILES):
                stage_u(i)
            for i in range(TILES - B_SKEW, TILES):
                stage_b(i)

            # epilogue: num_seg = (S + MAGIC - acc/V) - MAGIC; out = rs/num_seg
            nc.vector.scalar_tensor_tensor(
                out=w_sb[:], in0=acc_sb[:], scalar=-1.0 / V_INVALID, in1=carr[:],
                op0=alu.mult, op1=alu.add,
            )
            nc.vector.tensor_scalar(
                out=carr[:], in0=w_sb[:], scalar1=MAGIC, scalar2=None,
                op0=alu.subtract,
            )
            nc.vector.reciprocal(out=w_sb[:], in_=carr[:])
            nc.vector.tensor_tensor(
                out=carr[:], in0=rs_sb[:], in1=w_sb[:], op=alu.mult
            )
            nc.sync.dma_start(out=o_d[:, :], in_=carr[:])
    return _legalize_waits(nc)


def kernel(predictions: np.ndarray, targets: np.ndarray) -> np.ndarray:
    global _NC_CACHE
    from concourse.bass_utils import run_bass_kernel_spmd

    if _NC_CACHE is None:
        _NC_CACHE = _build_nc()
    nc = _NC_CACHE

    p = np.ascontiguousarray(predictions, dtype=np.float32)
    t = np.ascontiguousarray(targets, dtype=np.float32)
    in_maps = []
    for c in range(NCORES):
        sl = slice(c * ROWS_PER_CORE, (c + 1) * ROWS_PER_CORE)
        in_maps.append({"predictions": p[sl], "targets": t[sl]})
    res = run_bass_kernel_spmd(nc, in_maps, core_ids=list(range(NCORES)))
    total = 0.0
    for rmap in res.results:
        total += float(rmap["row_iou"].astype(np.float64).sum())
    return np.asarray(1.0 - total / B, dtype=np.float32)


# revision 15
# speedup vs baseline: 1.4349x; 1.0220x over previous
"""DepthIoULoss kernel for Trainium2 (Bass/Tile), data-parallel over 8 cores.

Math (per row, S segments; v[-1] treated as 0): with M = min(p, t) and
X = max(p, t) elementwise:
    inter_j = relu(M_j - X_{j-1});  union_j = X_j - M_{j-1};  iou = inter/union
Valid prefix: j <= stop_idx, where stop_idx = first index with t == 1.0.
row_iou = sum_valid iou_j / (stop_idx + 1);  loss = 1 - mean_rows(row_iou).

Device plan per [128, 2048] row-tile (only ops this walrus build accepts):
  ACT    tq  = t * K                      (K = 1e9, Copy w/ scale)
  DVE    cmx = exclusive-cummax(tq)       (tensor_tensor_scan max, shifted AP)
  DVE    M   = min(p, t);  X = max(p, t)  (min/max are DVE-only here;
                                           [128, S+1] tiles, zero column 0)
  GPSIMD i0  = M[:,1:] - X[:,:-1]
  GPSIMD u0  = X[:,1:] - M[:,:-1]
  DVE    u'  = max(cmx - 0.95K, u0), accum -> ia    (invalid lanes -> V=0.05K;
                                                     ia = n_invalid*V + O(1e3))
  ACT    lnu = Ln(u');  r = Exp(-lnu) = 1/u'        (invalid -> 2e-8)
  DVE    junk= max(i0,0) * r, accum -> rowsum       (relu fused here)
Epilogue on [128, 8]: num_seg = S - round(ia / V) (2^23 magic rounding),
row_iou = rowsum / num_seg -> DMA out. Host: loss = 1 - sum(row_iou) / B.

Manual software pipelining: engine queues run in EMISSION order, so the
um pass (which waits on GPSIMD's u0) is emitted one tile late and the
Ln/Exp/final passes two tiles late. This hides the Pool and ACT latency
behind the next tile's DVE work: sim went 140 us -> 104 us per core.

The masked lanes contribute |inter|/V <= 2048 * 2e-8 ~ 4e-5 absolute to a
rowsum of O(1..30): negligible. num_seg recovery is exact (error << 0.5).
"""

import numpy as np

B, S = 8192, 2048
NCORES = 8
ROWS_PER_CORE = B // NCORES  # 1024
TILES = ROWS_PER_CORE // 128  # 8

K_SCALE = np.float32(1.0e9)  # ACT Ln accurate to ~1e16; keep u' moderate
C_THRESH = np.float32(0.95) * K_SCALE
V_INVALID = float(np.float32(K_SCALE - np.float32(C_THRESH)))  # invalid-lane u'
MAGIC = 8388608.0  # 2**23: float add/sub rounds to nearest integer

UM_SKEW = 1  # um pass trails stage A by one tile
B_SKEW = 2  # ln/exp/final trail stage A by two tiles

_NC_CACHE = None

_RANGE_CLEAR_OPCODE = 176  # EVENT_SEMAPHORE_RANGE_CLEAR


def _legalize_waits(nc, maxw=1):
    """Make the Tile-generated module compatible with this walrus build.

    1. Drop tail EVENT_SEMAPHORE_RANGE_CLEAR InstISA ops (NRT re-initializes
       semaphore state per execution; this walrus rejects the encoding).
    2. Split instructions carrying more than `maxw` sync waits: excess waits
       move to carrier EventSemaphore nops inserted just before, same engine.
    """
    import concourse.mybir as mybir

    uid = [0]
    for fn in nc.m.functions:
        for blk in fn.blocks:
            lst = blk.instructions
            k = 0
            while k < len(lst):
                inst = lst[k]
                if (
                    type(inst).__name__ == "InstISA"
                    and getattr(inst, "isa_opcode", None) == _RANGE_CLEAR_OPCODE
                ):
                    si = inst.sync_info
                    if si is not None and (si.on_wait or si.on_update):
                        carrier = mybir.InstEventSemaphore(name=f"RCW-{uid[0]}")
                        uid[0] += 1
                        carrier.engine = inst.engine
                        carrier.sync_info = si
                        lst[k] = carrier
                        k += 1
                    else:
                        del lst[k]
                    continue
                si = inst.sync_info
                if si is not None and si.on_wait and len(si.on_wait) > maxw:
                    waits = list(si.on_wait)
                    extra, keep = waits[:-maxw], waits[-maxw:]
                    pos = k
                    for j in range(0, len(extra), maxw):
                        carrier = mybir.InstEventSemaphore(name=f"EVW-{uid[0]}")
                        uid[0] += 1
                        carrier.engine = inst.engine
                        carrier.sync_info = mybir.SyncInfo(
                            on_wait=extra[j : j + maxw], on_update=[]
                        )
                        lst.insert(pos, carrier)
                        pos += 1
                        k += 1
                    inst.sync_info = mybir.SyncInfo(
                        on_wait=keep, on_update=list(si.on_update)
                    )
                k += 1
    return nc


def _build_nc():
    import concourse.bass as bass
    import concourse.mybir as mybir
    from concourse.tile import TileContext

    f32 = mybir.dt.float32
    alu = mybir.AluOpType
    act = mybir.ActivationFunctionType

    nc = bass.Bass()
    p_d = nc.dram_tensor("predictions", [ROWS_PER_CORE, S], f32, kind="ExternalInput")
    t_d = nc.dram_tensor("targets", [ROWS_PER_CORE, S], f32, kind="ExternalInput")
    o_d = nc.dram_tensor("row_iou", [128, TILES], f32, kind="ExternalOutput")

    with TileContext(nc) as tc:
        with (
            tc.tile_pool(name="io", bufs=2) as iop,
            tc.tile_pool(name="geom", bufs=2) as gp,
            tc.tile_pool(name="cmxp", bufs=3) as cp,
            tc.tile_pool(name="i0p", bufs=2) as i0p,
            tc.tile_pool(name="u0p", bufs=2) as u0p,
            tc.tile_pool(name="ump", bufs=3) as ump,
            tc.tile_pool(name="uch", bufs=2) as up,
            tc.tile_pool(name="smp", bufs=1) as smp,
        ):
            acc_sb = smp.tile([128, TILES], f32, tag="acc")
            rs_sb = smp.tile([128, TILES], f32, tag="rs")
            carr = smp.tile([128, TILES], f32, tag="carr")
            nc.vector.memset(carr[:], float(S) + MAGIC)
            w_sb = smp.tile([128, TILES], f32, tag="w")

            st_a = {}
            st_u = {}

            H = S // 2

            def stage_a(i):
                rows = slice(i * 128, (i + 1) * 128)
                p = iop.tile([128, S], f32, tag="p")
                t = iop.tile([128, S], f32, tag="t")
                tq = gp.tile([128, S], f32, tag="tq")
                cmx = cp.tile([128, S], f32, tag="cmx")
                M = gp.tile([128, S + 1], f32, tag="M")
                X = gp.tile([128, S + 1], f32, tag="X")
                nc.scalar.memzero(cmx[:, 0:1])
                nc.scalar.memzero(M[:, 0:1])
                nc.gpsimd.memset(X[:, 0:1], 0.0)
                if i == 0:
                    # split tile 0 into column halves: DVE ramps up ~5us sooner
                    nc.sync.dma_start(out=t[:, 0:H], in_=t_d[rows, 0:H])
                    nc.sync.dma_start(out=p[:, 0:H], in_=p_d[rows, 0:H])
                    nc.sync.dma_start(out=t[:, H:S], in_=t_d[rows, H:S])
                    nc.sync.dma_start(out=p[:, H:S], in_=p_d[rows, H:S])
                    nc.scalar.activation(out=tq[:, 0:H], in_=t[:, 0:H],
                                         func=act.Copy, scale=float(K_SCALE))
                    nc.vector.tensor_tensor_scan(
                        out=cmx[:, 1 : H + 1], data0=tq[:, 0:H], data1=tq[:, 0:H],
                        initial=0.0, op0=alu.max, op1=alu.bypass)
                    nc.vector.tensor_tensor(
                        out=M[:, 1 : H + 1], in0=p[:, 0:H], in1=t[:, 0:H],
                        op=alu.min)
                    nc.vector.tensor_tensor(
                        out=X[:, 1 : H + 1], in0=p[:, 0:H], in1=t[:, 0:H],
                        op=alu.max)
                    nc.scalar.activation(out=tq[:, H:S], in_=t[:, H:S],
                                         func=act.Copy, scale=float(K_SCALE))
                    nc.vector.tensor_tensor_scan(
                        out=cmx[:, H + 1 : S], data0=tq[:, H : S - 1],
                        data1=tq[:, H : S - 1],
                        initial=cmx[:, H : H + 1], op0=alu.max, op1=alu.bypass)
                    nc.vector.tensor_tensor(
                        out=M[:, H + 1 : S + 1], in0=p[:, H:S], in1=t[:, H:S],
                        op=alu.min)
                    nc.vector.tensor_tensor(
                        out=X[:, H + 1 : S + 1], in0=p[:, H:S], in1=t[:, H:S],
                        op=alu.max)
                else:
                    nc.sync.dma_start(out=t[:], in_=t_d[rows, :])
                    nc.sync.dma_start(out=p[:], in_=p_d[rows, :])
                    nc.scalar.activation(
                        out=tq[:], in_=t[:], func=act.Copy, scale=float(K_SCALE)
                    )
                    nc.vector.tensor_tensor_scan(
                        out=cmx[:, 1:S],
                        data0=tq[:, 0 : S - 1],
                        data1=tq[:, 0 : S - 1],
                        initial=0.0,
                        op0=alu.max,
                        op1=alu.bypass,
                    )
                    nc.vector.tensor_tensor(
                        out=M[:, 1 : S + 1], in0=p[:], in1=t[:], op=alu.min
                    )
                    nc.vector.tensor_tensor(
                        out=X[:, 1 : S + 1], in0=p[:], in1=t[:], op=alu.max
                    )

                i0 = i0p.tile([128, S], f32, tag="i0")
                nc.gpsimd.tensor_tensor(
                    out=i0[:], in0=M[:, 1 : S + 1], in1=X[:, 0:S], op=alu.subtract
                )
                u0 = u0p.tile([128, S], f32, tag="u0")
                nc.gpsimd.tensor_tensor(
                    out=u0[:], in0=X[:, 1 : S + 1], in1=M[:, 0:S], op=alu.subtract
                )
                st_a[i] = (i0, u0, cmx)

            def stage_u(i):
                i0, u0, cmx = st_a.pop(i)
                um = ump.tile([128, S], f32, tag="um")
                nc.vector.scalar_tensor_tensor(
                    out=um[:],
                    in0=cmx[:],
                    scalar=float(C_THRESH),
                    in1=u0[:],
                    op0=alu.subtract,
                    op1=alu.max,
                    accum_out=acc_sb[:, i : i + 1],
                )
                st_u[i] = (i0, um)

            def stage_b(i):
                i0, um = st_u.pop(i)
                lnu = up.tile([128, S], f32, tag="lnu")
                nc.scalar.activation(out=lnu[:], in_=um[:], func=act.Ln)
                r = up.tile([128, S], f32, tag="r")
                nc.scalar.activation(out=r[:], in_=lnu[:], func=act.Exp, scale=-1.0)
                nc.vector.scalar_tensor_tensor(
                    out=um[:],
                    in0=i0[:],
                    scalar=0.0,
                    in1=r[:],
                    op0=alu.max,
                    op1=alu.mult,
                    accum_out=rs_sb[:, i : i + 1],
                )

            for i in range(TILES):
                stage_a(i)
                if i >= UM_SKEW:
                    stage_u(i - UM_SKEW)
                if i >= B_SKEW:
                    stage_b(i - B_SKEW)
            for i in range(TILES - UM_SKEW, TILES):
                stage_u(i)
            for i in range(TILES - B_SKEW, TILES):
                stage_b(i)

            # epilogue: num_seg = (S + MAGIC - acc/V) - MAGIC; out = rs/num_seg
            nc.vector.scalar_tensor_tensor(
                out=w_sb[:], in0=acc_sb[:], scalar=-1.0 / V_INVALID, in1=carr[:],
                op0=alu.mult, op1=alu.add,
            )
            nc.vector.tensor_scalar(
                out=carr[:], in0=w_sb[:], scalar1=MAGIC, scalar2=None,
                op0=alu.subtract,
            )
            nc.vector.reciprocal(out=w_sb[:], in_=carr[:])
            nc.vector.tensor_tensor(
                out=carr[:], in0=rs_sb[:], in1=w_sb[:], op=alu.mult
            )
            nc.sync.dma_start(out=o_d[:, :], in_=carr[:])
    return _legalize_waits(nc)


def kernel(predictions: np.ndarray, targets: np.ndarray) -> np.ndarray:
    global _NC_CACHE
    from concourse.bass_utils import run_bass_kernel_spmd

    if _NC_CACHE is None:
        _NC_CACHE = _build_nc()
    nc = _NC_CACHE

    p = np.ascontiguousarray(predictions, dtype=np.float32)
    t = np.ascontiguousarray(targets, dtype=np.float32)
    in_maps = []
    for c in range(NCORES):
        sl = slice(c * ROWS_PER_CORE, (c + 1) * ROWS_PER_CORE)
        in_maps.append({"predictions": p[sl], "targets": t[sl]})
    res = run_bass_kernel_spmd(nc, in_maps, core_ids=list(range(NCORES)))
    total = 0.0
    for rmap in res.results:
        total += float(rmap["row_iou"].astype(np.float64).sum())
    return np.asarray(1.0 - total / B, dtype=np.float32)


# revision 16
# speedup vs baseline: 1.4505x; 1.0109x over previous
"""DepthIoULoss kernel for Trainium2 (Bass/Tile), data-parallel over 8 cores.

Math (per row, S segments; v[-1] treated as 0): with M = min(p, t) and
X = max(p, t) elementwise:
    inter_j = relu(M_j - X_{j-1});  union_j = X_j - M_{j-1};  iou = inter/union
Valid prefix: j <= stop_idx, where stop_idx = first index with t == 1.0.
row_iou = sum_valid iou_j / (stop_idx + 1);  loss = 1 - mean_rows(row_iou).

Device plan per [128, 2048] row-tile (only ops this walrus build accepts):
  ACT    tq  = t * K                      (K = 1e9, Copy w/ scale)
  DVE    cmx = exclusive-cummax(tq)       (tensor_tensor_scan max, shifted AP)
  DVE    M   = min(p, t);  X = max(p, t)  (min/max are DVE-only here;
                                           [128, S+1] tiles, zero column 0)
  GPSIMD i0  = M[:,1:] - X[:,:-1]
  GPSIMD u0  = X[:,1:] - M[:,:-1]
  DVE    u'  = max(cmx - 0.95K, u0), accum -> ia    (invalid lanes -> V=0.05K;
                                                     ia = n_invalid*V + O(1e3))
  ACT    lnu = Ln(u');  r = Exp(-lnu) = 1/u'        (invalid -> 2e-8)
  DVE    junk= max(i0,0) * r, accum -> rowsum       (relu fused here)
Epilogue on [128, 8]: num_seg = S - round(ia / V) (2^23 magic rounding),
row_iou = rowsum / num_seg -> DMA out. Host: loss = 1 - sum(row_iou) / B.

Manual software pipelining: engine queues run in EMISSION order, so the
um pass (which waits on GPSIMD's u0) is emitted one tile late and the
Ln/Exp/final passes two tiles late. This hides the Pool and ACT latency
behind the next tile's DVE work: sim went 140 us -> 104 us per core.

The masked lanes contribute |inter|/V <= 2048 * 2e-8 ~ 4e-5 absolute to a
rowsum of O(1..30): negligible. num_seg recovery is exact (error << 0.5).
"""

import numpy as np

B, S = 8192, 2048
NCORES = 8
ROWS_PER_CORE = B // NCORES  # 1024
TILES = ROWS_PER_CORE // 128  # 8

K_SCALE = np.float32(1.0e9)  # ACT Ln accurate to ~1e16; keep u' moderate
C_THRESH = np.float32(0.95) * K_SCALE
V_INVALID = float(np.float32(K_SCALE - np.float32(C_THRESH)))  # invalid-lane u'
MAGIC = 8388608.0  # 2**23: float add/sub rounds to nearest integer

UM_SKEW = 1  # um pass trails stage A by one tile
B_SKEW = 2  # ln/exp/final trail stage A by two tiles

_NC_CACHE = None

_RANGE_CLEAR_OPCODE = 176  # EVENT_SEMAPHORE_RANGE_CLEAR


def _legalize_waits(nc, maxw=1):
    """Make the Tile-generated module compatible with this walrus build.

    1. Drop tail EVENT_SEMAPHORE_RANGE_CLEAR InstISA ops (NRT re-initializes
       semaphore state per execution; this walrus rejects the encoding).
    2. Split instructions carrying more than `maxw` sync waits: excess waits
       move to carrier EventSemaphore nops inserted just before, same engine.
    """
    import concourse.mybir as mybir

    uid = [0]
    for fn in nc.m.functions:
        for blk in fn.blocks:
            lst = blk.instructions
            k = 0
            while k < len(lst):
                inst = lst[k]
                if (
                    type(inst).__name__ == "InstISA"
                    and getattr(inst, "isa_opcode", None) == _RANGE_CLEAR_OPCODE
                ):
                    si = inst.sync_info
                    if si is not None and (si.on_wait or si.on_update):
                        carrier = mybir.InstEventSemaphore(name=f"RCW-{uid[0]}")
                        uid[0] += 1
                        carrier.engine = inst.engine
                        carrier.sync_info = si
                        lst[k] = carrier
                        k += 1
                    else:
                        del lst[k]
                    continue
                si = inst.sync_info
                if si is not None and si.on_wait and len(si.on_wait) > maxw:
                    waits = list(si.on_wait)
                    extra, keep = waits[:-maxw], waits[-maxw:]
                    pos = k
                    for j in range(0, len(extra), maxw):
                        carrier = mybir.InstEventSemaphore(name=f"EVW-{uid[0]}")
                        uid[0] += 1
                        carrier.engine = inst.engine
                        carrier.sync_info = mybir.SyncInfo(
                            on_wait=extra[j : j + maxw], on_update=[]
                        )
                        lst.insert(pos, carrier)
                        pos += 1
                        k += 1
                    inst.sync_info = mybir.SyncInfo(
                        on_wait=keep, on_update=list(si.on_update)
                    )
                k += 1
    return nc


def _build_nc():
    import concourse.bass as bass
    import concourse.mybir as mybir
    from concourse.tile import TileContext

    f32 = mybir.dt.float32
    alu = mybir.AluOpType
    act = mybir.ActivationFunctionType

    nc = bass.Bass()
    p_d = nc.dram_tensor("predictions", [ROWS_PER_CORE, S], f32, kind="ExternalInput")
    t_d = nc.dram_tensor("targets", [ROWS_PER_CORE, S], f32, kind="ExternalInput")
    o_d = nc.dram_tensor("row_iou", [128, TILES], f32, kind="ExternalOutput")

    with TileContext(nc) as tc:
        with (
            tc.tile_pool(name="io", bufs=2) as iop,
            tc.tile_pool(name="geom", bufs=2) as gp,
            tc.tile_pool(name="cmxp", bufs=3) as cp,
            tc.tile_pool(name="i0p", bufs=2) as i0p,
            tc.tile_pool(name="u0p", bufs=2) as u0p,
            tc.tile_pool(name="ump", bufs=3) as ump,
            tc.tile_pool(name="uch", bufs=2) as up,
            tc.tile_pool(name="smp", bufs=1) as smp,
        ):
            acc_sb = smp.tile([128, TILES], f32, tag="acc")
            rs_sb = smp.tile([128, TILES], f32, tag="rs")
            carr = smp.tile([128, TILES], f32, tag="carr")
            nc.vector.memset(carr[:], float(S) + MAGIC)
            w_sb = smp.tile([128, TILES], f32, tag="w")

            st_a = {}
            st_u = {}

            def stage_a(i):
                rows = slice(i * 128, (i + 1) * 128)
                p = iop.tile([128, S], f32, tag="p")
                t = iop.tile([128, S], f32, tag="t")
                tq = gp.tile([128, S], f32, tag="tq")
                cmx = cp.tile([128, S], f32, tag="cmx")
                M = gp.tile([128, S + 1], f32, tag="M")
                X = gp.tile([128, S + 1], f32, tag="X")
                nc.scalar.memzero(cmx[:, 0:1])
                nc.scalar.memzero(M[:, 0:1])
                nc.gpsimd.memset(X[:, 0:1], 0.0)
                # tile 0 is processed in column quarters so DVE ramps up while
                # the rest of the data is still in flight (chained scan).
                nparts = 4 if i == 0 else 1
                Hc = S // nparts
                for k in range(nparts):
                    a, b = k * Hc, (k + 1) * Hc
                    nc.sync.dma_start(out=t[:, a:b], in_=t_d[rows, a:b])
                    nc.sync.dma_start(out=p[:, a:b], in_=p_d[rows, a:b])
                    nc.scalar.activation(
                        out=tq[:, a:b], in_=t[:, a:b], func=act.Copy,
                        scale=float(K_SCALE),
                    )
                    last = k == nparts - 1
                    nc.vector.tensor_tensor_scan(
                        out=cmx[:, a + 1 : (b if last else b + 1)],
                        data0=tq[:, a : (b - 1 if last else b)],
                        data1=tq[:, a : (b - 1 if last else b)],
                        initial=(0.0 if k == 0 else cmx[:, a : a + 1]),
                        op0=alu.max,
                        op1=alu.bypass,
                    )
                    nc.vector.tensor_tensor(
                        out=M[:, a + 1 : b + 1], in0=p[:, a:b], in1=t[:, a:b],
                        op=alu.min,
                    )
                    nc.vector.tensor_tensor(
                        out=X[:, a + 1 : b + 1], in0=p[:, a:b], in1=t[:, a:b],
                        op=alu.max,
                    )
                i0 = i0p.tile([128, S], f32, tag="i0")
                nc.gpsimd.tensor_tensor(
                    out=i0[:], in0=M[:, 1 : S + 1], in1=X[:, 0:S], op=alu.subtract
                )
                u0 = u0p.tile([128, S], f32, tag="u0")
                nc.gpsimd.tensor_tensor(
                    out=u0[:], in0=X[:, 1 : S + 1], in1=M[:, 0:S], op=alu.subtract
                )
                st_a[i] = (i0, u0, cmx)

            def stage_u(i):
                i0, u0, cmx = st_a.pop(i)
                um = ump.tile([128, S], f32, tag="um")
                nc.vector.scalar_tensor_tensor(
                    out=um[:],
                    in0=cmx[:],
                    scalar=float(C_THRESH),
                    in1=u0[:],
                    op0=alu.subtract,
                    op1=alu.max,
                    accum_out=acc_sb[:, i : i + 1],
                )
                st_u[i] = (i0, um)

            def stage_b(i, split=False):
                i0, um = st_u.pop(i)
                lnu = up.tile([128, S], f32, tag="lnu")
                r = up.tile([128, S], f32, tag="r")
                if split:
                    # last tile: halve the Ln/Exp/final chain to shrink the
                    # serial drain tail; partial row-sums add up afterwards.
                    Hh = S // 2
                    nc.scalar.activation(out=lnu[:, 0:Hh], in_=um[:, 0:Hh],
                                         func=act.Ln)
                    nc.scalar.activation(out=r[:, 0:Hh], in_=lnu[:, 0:Hh],
                                         func=act.Exp, scale=-1.0)
                    nc.scalar.activation(out=lnu[:, Hh:S], in_=um[:, Hh:S],
                                         func=act.Ln)
                    nc.scalar.activation(out=r[:, Hh:S], in_=lnu[:, Hh:S],
                                         func=act.Exp, scale=-1.0)
                    junk = ump.tile([128, S], f32, tag="um")
                    nc.vector.scalar_tensor_tensor(
                        out=junk[:, 0:Hh], in0=i0[:, 0:Hh], scalar=0.0,
                        in1=r[:, 0:Hh], op0=alu.max, op1=alu.mult,
                        accum_out=w_sb[:, i : i + 1],
                    )
                    nc.vector.scalar_tensor_tensor(
                        out=junk[:, Hh:S], in0=i0[:, Hh:S], scalar=0.0,
                        in1=r[:, Hh:S], op0=alu.max, op1=alu.mult,
                        accum_out=rs_sb[:, i : i + 1],
                    )
                    nc.vector.tensor_tensor(
                        out=rs_sb[:, i : i + 1], in0=rs_sb[:, i : i + 1],
                        in1=w_sb[:, i : i + 1], op=alu.add,
                    )
                else:
                    nc.scalar.activation(out=lnu[:], in_=um[:], func=act.Ln)
                    nc.scalar.activation(out=r[:], in_=lnu[:], func=act.Exp,
                                         scale=-1.0)
                    nc.vector.scalar_tensor_tensor(
                        out=um[:],
                        in0=i0[:],
                        scalar=0.0,
                        in1=r[:],
                        op0=alu.max,
                        op1=alu.mult,
                        accum_out=rs_sb[:, i : i + 1],
                    )

            for i in range(TILES):
                stage_a(i)
                if i >= UM_SKEW:
                    stage_u(i - UM_SKEW)
                if i >= B_SKEW:
                    stage_b(i - B_SKEW)
            for i in range(TILES - UM_SKEW, TILES):
                stage_u(i)
            for i in range(TILES - B_SKEW, TILES):
                stage_b(i, split=(i == TILES - 1))

            # epilogue: num_seg = (S + MAGIC - acc/V) - MAGIC; out = rs/num_seg
            nc.vector.scalar_tensor_tensor(
                out=w_sb[:], in0=acc_sb[:], scalar=-1.0 / V_INVALID, in1=carr[:],
                op0=alu.mult, op1=alu.add,
            )
            nc.vector.tensor_scalar(
                out=carr[:], in0=w_sb[:], scalar1=MAGIC, scalar2=None,
                op0=alu.subtract,
            )
            nc.vector.reciprocal(out=w_sb[:], in_=carr[:])
            nc.vector.tensor_tensor(
                out=carr[:], in0=rs_sb[:], in1=w_sb[:], op=alu.mult
            )
            nc.sync.dma_start(out=o_d[:, :], in_=carr[:])
    return _legalize_waits(nc)


def kernel(predictions: np.ndarray, targets: np.ndarray) -> np.ndarray:
    global _NC_CACHE
    from concourse.bass_utils import run_bass_kernel_spmd

    if _NC_CACHE is None:
        _NC_CACHE = _build_nc()
    nc = _NC_CACHE

    p = np.ascontiguousarray(predictions, dtype=np.float32)
    t = np.ascontiguousarray(targets, dtype=np.float32)
    in_maps = []
    for c in range(NCORES):
        sl = slice(c * ROWS_PER_CORE, (c + 1) * ROWS_PER_CORE)
        in_maps.append({"predictions": p[sl], "targets": t[sl]})
    res = run_bass_kernel_spmd(nc, in_maps, core_ids=list(range(NCORES)))
    total = 0.0
    for rmap in res.results:
        total += float(rmap["row_iou"].astype(np.float64).sum())
    return np.asarray(1.0 - total / B, dtype=np.float32)


# revision 17
# speedup vs baseline: 1.4579x; 1.0051x over previous
"""DepthIoULoss kernel for Trainium2 (Bass/Tile), data-parallel over 8 cores.

Math (per row, S segments; v[-1] treated as 0): with M = min(p, t) and
X = max(p, t) elementwise:
    inter_j = relu(M_j - X_{j-1});  union_j = X_j - M_{j-1};  iou = inter/union
Valid prefix: j <= stop_idx, where stop_idx = first index with t == 1.0.
row_iou = sum_valid iou_j / (stop_idx + 1);  loss = 1 - mean_rows(row_iou).

Device plan per [128, 2048] row-tile (only ops this walrus build accepts):
  ACT    tq  = t * K                      (K = 1e9, Copy w/ scale)
  DVE    cmx = exclusive-cummax(tq)       (tensor_tensor_scan max, shifted AP)
  DVE    M   = min(p, t);  X = max(p, t)  (min/max are DVE-only here;
                                           [128, S+1] tiles, zero column 0)
  GPSIMD i0  = M[:,1:] - X[:,:-1]
  GPSIMD u0  = X[:,1:] - M[:,:-1]
  DVE    u'  = max(cmx - 0.95K, u0), accum -> ia    (invalid lanes -> V=0.05K;
                                                     ia = n_invalid*V + O(1e3))
  ACT    lnu = Ln(u');  r = Exp(-lnu) = 1/u'        (invalid -> 2e-8)
  DVE    junk= max(i0,0) * r, accum -> rowsum       (relu fused here)
Epilogue on [128, 8]: num_seg = S - round(ia / V) (2^23 magic rounding),
row_iou = rowsum / num_seg -> DMA out. Host: loss = 1 - sum(row_iou) / B.

Manual software pipelining: engine queues run in EMISSION order, so the
um pass (which waits on GPSIMD's u0) is emitted one tile late and the
Ln/Exp/final passes two tiles late. This hides the Pool and ACT latency
behind the next tile's DVE work: sim went 140 us -> 104 us per core.

The masked lanes contribute |inter|/V <= 2048 * 2e-8 ~ 4e-5 absolute to a
rowsum of O(1..30): negligible. num_seg recovery is exact (error << 0.5).
"""

import numpy as np

B, S = 8192, 2048
NCORES = 8
ROWS_PER_CORE = B // NCORES  # 1024
TILES = ROWS_PER_CORE // 128  # 8

K_SCALE = np.float32(1.0e9)  # ACT Ln accurate to ~1e16; keep u' moderate
C_THRESH = np.float32(0.95) * K_SCALE
V_INVALID = float(np.float32(K_SCALE - np.float32(C_THRESH)))  # invalid-lane u'
MAGIC = 8388608.0  # 2**23: float add/sub rounds to nearest integer

UM_SKEW = 1  # um pass trails stage A by one tile
B_SKEW = 2  # ln/exp/final trail stage A by two tiles

_NC_CACHE = None

_RANGE_CLEAR_OPCODE = 176  # EVENT_SEMAPHORE_RANGE_CLEAR


def _legalize_waits(nc, maxw=1):
    """Make the Tile-generated module compatible with this walrus build.

    1. Drop tail EVENT_SEMAPHORE_RANGE_CLEAR InstISA ops (NRT re-initializes
       semaphore state per execution; this walrus rejects the encoding).
    2. Split instructions carrying more than `maxw` sync waits: excess waits
       move to carrier EventSemaphore nops inserted just before, same engine.
    """
    import concourse.mybir as mybir

    uid = [0]
    for fn in nc.m.functions:
        for blk in fn.blocks:
            lst = blk.instructions
            k = 0
            while k < len(lst):
                inst = lst[k]
                if (
                    type(inst).__name__ == "InstISA"
                    and getattr(inst, "isa_opcode", None) == _RANGE_CLEAR_OPCODE
                ):
                    si = inst.sync_info
                    if si is not None and (si.on_wait or si.on_update):
                        carrier = mybir.InstEventSemaphore(name=f"RCW-{uid[0]}")
                        uid[0] += 1
                        carrier.engine = inst.engine
                        carrier.sync_info = si
                        lst[k] = carrier
                        k += 1
                    else:
                        del lst[k]
                    continue
                si = inst.sync_info
                if si is not None and si.on_wait and len(si.on_wait) > maxw:
                    waits = list(si.on_wait)
                    extra, keep = waits[:-maxw], waits[-maxw:]
                    pos = k
                    for j in range(0, len(extra), maxw):
                        carrier = mybir.InstEventSemaphore(name=f"EVW-{uid[0]}")
                        uid[0] += 1
                        carrier.engine = inst.engine
                        carrier.sync_info = mybir.SyncInfo(
                            on_wait=extra[j : j + maxw], on_update=[]
                        )
                        lst.insert(pos, carrier)
                        pos += 1
                        k += 1
                    inst.sync_info = mybir.SyncInfo(
                        on_wait=keep, on_update=list(si.on_update)
                    )
                k += 1
    return nc


def _build_nc():
    import concourse.bass as bass
    import concourse.mybir as mybir
    from concourse.tile import TileContext

    f32 = mybir.dt.float32
    alu = mybir.AluOpType
    act = mybir.ActivationFunctionType

    nc = bass.Bass()
    p_d = nc.dram_tensor("predictions", [ROWS_PER_CORE, S], f32, kind="ExternalInput")
    t_d = nc.dram_tensor("targets", [ROWS_PER_CORE, S], f32, kind="ExternalInput")
    o_d = nc.dram_tensor("row_iou", [128, TILES], f32, kind="ExternalOutput")

    with TileContext(nc) as tc:
        with (
            tc.tile_pool(name="io", bufs=2) as iop,
            tc.tile_pool(name="geom", bufs=2) as gp,
            tc.tile_pool(name="cmxp", bufs=3) as cp,
            tc.tile_pool(name="i0p", bufs=2) as i0p,
            tc.tile_pool(name="u0p", bufs=2) as u0p,
            tc.tile_pool(name="ump", bufs=3) as ump,
            tc.tile_pool(name="uch", bufs=2) as up,
            tc.tile_pool(name="smp", bufs=1) as smp,
        ):
            acc_sb = smp.tile([128, TILES], f32, tag="acc")
            rs_sb = smp.tile([128, TILES], f32, tag="rs")
            carr = smp.tile([128, TILES], f32, tag="carr")
            nc.vector.memset(carr[:], float(S) + MAGIC)
            w_sb = smp.tile([128, TILES], f32, tag="w")
            w2_sb = smp.tile([128, TILES], f32, tag="w2")

            st_a = {}
            st_u = {}

            def stage_a(i):
                rows = slice(i * 128, (i + 1) * 128)
                p = iop.tile([128, S], f32, tag="p")
                t = iop.tile([128, S], f32, tag="t")
                tq = gp.tile([128, S], f32, tag="tq")
                cmx = cp.tile([128, S], f32, tag="cmx")
                M = gp.tile([128, S + 1], f32, tag="M")
                X = gp.tile([128, S + 1], f32, tag="X")
                nc.scalar.memzero(cmx[:, 0:1])
                nc.scalar.memzero(M[:, 0:1])
                nc.gpsimd.memset(X[:, 0:1], 0.0)
                # tile 0 is processed in column quarters so DVE ramps up while
                # the rest of the data is still in flight (chained scan).
                nparts = 4 if i == 0 else 1
                Hc = S // nparts
                for k in range(nparts):
                    a, b = k * Hc, (k + 1) * Hc
                    nc.sync.dma_start(out=t[:, a:b], in_=t_d[rows, a:b])
                    nc.sync.dma_start(out=p[:, a:b], in_=p_d[rows, a:b])
                    nc.scalar.activation(
                        out=tq[:, a:b], in_=t[:, a:b], func=act.Copy,
                        scale=float(K_SCALE),
                    )
                    last = k == nparts - 1
                    nc.vector.tensor_tensor_scan(
                        out=cmx[:, a + 1 : (b if last else b + 1)],
                        data0=tq[:, a : (b - 1 if last else b)],
                        data1=tq[:, a : (b - 1 if last else b)],
                        initial=(0.0 if k == 0 else cmx[:, a : a + 1]),
                        op0=alu.max,
                        op1=alu.bypass,
                    )
                    nc.vector.tensor_tensor(
                        out=M[:, a + 1 : b + 1], in0=p[:, a:b], in1=t[:, a:b],
                        op=alu.min,
                    )
                    nc.vector.tensor_tensor(
                        out=X[:, a + 1 : b + 1], in0=p[:, a:b], in1=t[:, a:b],
                        op=alu.max,
                    )
                i0 = i0p.tile([128, S], f32, tag="i0")
                nc.gpsimd.tensor_tensor(
                    out=i0[:], in0=M[:, 1 : S + 1], in1=X[:, 0:S], op=alu.subtract
                )
                u0 = u0p.tile([128, S], f32, tag="u0")
                nc.gpsimd.tensor_tensor(
                    out=u0[:], in0=X[:, 1 : S + 1], in1=M[:, 0:S], op=alu.subtract
                )
                st_a[i] = (i0, u0, cmx)

            def stage_u(i, split=False):
                i0, u0, cmx = st_a.pop(i)
                um = ump.tile([128, S], f32, tag="um")
                if split:
                    # last tile: halve the mask pass so Ln can start sooner;
                    # the two partial accums add up to the same ia.
                    Hh = S // 2
                    nc.vector.scalar_tensor_tensor(
                        out=um[:, 0:Hh], in0=cmx[:, 0:Hh],
                        scalar=float(C_THRESH), in1=u0[:, 0:Hh],
                        op0=alu.subtract, op1=alu.max,
                        accum_out=w2_sb[:, i : i + 1],
                    )
                    nc.vector.scalar_tensor_tensor(
                        out=um[:, Hh:S], in0=cmx[:, Hh:S],
                        scalar=float(C_THRESH), in1=u0[:, Hh:S],
                        op0=alu.subtract, op1=alu.max,
                        accum_out=acc_sb[:, i : i + 1],
                    )
                    nc.vector.tensor_tensor(
                        out=acc_sb[:, i : i + 1], in0=acc_sb[:, i : i + 1],
                        in1=w2_sb[:, i : i + 1], op=alu.add,
                    )
                else:
                    nc.vector.scalar_tensor_tensor(
                        out=um[:],
                        in0=cmx[:],
                        scalar=float(C_THRESH),
                        in1=u0[:],
                        op0=alu.subtract,
                        op1=alu.max,
                        accum_out=acc_sb[:, i : i + 1],
                    )
                st_u[i] = (i0, um)

            def stage_b(i, split=False):
                i0, um = st_u.pop(i)
                lnu = up.tile([128, S], f32, tag="lnu")
                r = up.tile([128, S], f32, tag="r")
                if split:
                    # last tile: halve the Ln/Exp/final chain to shrink the
                    # serial drain tail; partial row-sums add up afterwards.
                    Hh = S // 2
                    nc.scalar.activation(out=lnu[:, 0:Hh], in_=um[:, 0:Hh],
                                         func=act.Ln)
                    nc.scalar.activation(out=r[:, 0:Hh], in_=lnu[:, 0:Hh],
                                         func=act.Exp, scale=-1.0)
                    nc.scalar.activation(out=lnu[:, Hh:S], in_=um[:, Hh:S],
                                         func=act.Ln)
                    nc.scalar.activation(out=r[:, Hh:S], in_=lnu[:, Hh:S],
                                         func=act.Exp, scale=-1.0)
                    junk = ump.tile([128, S], f32, tag="um")
                    nc.vector.scalar_tensor_tensor(
                        out=junk[:, 0:Hh], in0=i0[:, 0:Hh], scalar=0.0,
                        in1=r[:, 0:Hh], op0=alu.max, op1=alu.mult,
                        accum_out=w_sb[:, i : i + 1],
                    )
                    nc.vector.scalar_tensor_tensor(
                        out=junk[:, Hh:S], in0=i0[:, Hh:S], scalar=0.0,
                        in1=r[:, Hh:S], op0=alu.max, op1=alu.mult,
                        accum_out=rs_sb[:, i : i + 1],
                    )
                    nc.vector.tensor_tensor(
                        out=rs_sb[:, i : i + 1], in0=rs_sb[:, i : i + 1],
                        in1=w_sb[:, i : i + 1], op=alu.add,
                    )
                else:
                    nc.scalar.activation(out=lnu[:], in_=um[:], func=act.Ln)
                    nc.scalar.activation(out=r[:], in_=lnu[:], func=act.Exp,
                                         scale=-1.0)
                    nc.vector.scalar_tensor_tensor(
                        out=um[:],
                        in0=i0[:],
                        scalar=0.0,
                        in1=r[:],
                        op0=alu.max,
                        op1=alu.mult,
                        accum_out=rs_sb[:, i : i + 1],
                    )

            for i in range(TILES):
                stage_a(i)
                if i >= UM_SKEW:
                    stage_u(i - UM_SKEW)
                if i >= B_SKEW:
                    stage_b(i - B_SKEW)
            for i in range(TILES - UM_SKEW, TILES):
                stage_u(i, split=(i == TILES - 1))
            for i in range(TILES - B_SKEW, TILES):
                stage_b(i, split=(i == TILES - 1))

            # epilogue: num_seg = (S + MAGIC - acc/V) - MAGIC; out = rs/num_seg
            nc.vector.scalar_tensor_tensor(
                out=w_sb[:], in0=acc_sb[:], scalar=-1.0 / V_INVALID, in1=carr[:],
                op0=alu.mult, op1=alu.add,
            )
            nc.vector.tensor_scalar(
                out=carr[:], in0=w_sb[:], scalar1=MAGIC, scalar2=None,
                op0=alu.subtract,
            )
            nc.vector.reciprocal(out=w_sb[:], in_=carr[:])
            nc.vector.tensor_tensor(
                out=carr[:], in0=rs_sb[:], in1=w_sb[:], op=alu.mult
            )
            nc.sync.dma_start(out=o_d[:, :], in_=carr[:])
    return _legalize_waits(nc)


def kernel(predictions: np.ndarray, targets: np.ndarray) -> np.ndarray:
    global _NC_CACHE
    from concourse.bass_utils import run_bass_kernel_spmd

    if _NC_CACHE is None:
        _NC_CACHE = _build_nc()
    nc = _NC_CACHE

    p = np.ascontiguousarray(predictions, dtype=np.float32)
    t = np.ascontiguousarray(targets, dtype=np.float32)
    in_maps = []
    for c in range(NCORES):
        sl = slice(c * ROWS_PER_CORE, (c + 1) * ROWS_PER_CORE)
        in_maps.append({"predictions": p[sl], "targets": t[sl]})
    res = run_bass_kernel_spmd(nc, in_maps, core_ids=list(range(NCORES)))
    total = 0.0
    for rmap in res.results:
        total += float(rmap["row_iou"].astype(np.float64).sum())
    return np.asarray(1.0 - total / B, dtype=np.float32)


# revision 18
# speedup vs baseline: 1.4686x; 1.0073x over previous
"""DepthIoULoss kernel for Trainium2 (Bass/Tile), data-parallel over 8 cores.

Math (per row, S segments; v[-1] treated as 0): with M = min(p, t) and
X = max(p, t) elementwise:
    inter_j = relu(M_j - X_{j-1});  union_j = X_j - M_{j-1};  iou = inter/union
Valid prefix: j <= stop_idx, where stop_idx = first index with t == 1.0.
row_iou = sum_valid iou_j / (stop_idx + 1);  loss = 1 - mean_rows(row_iou).

Device plan per [128, 2048] row-tile (only ops this walrus build accepts):
  ACT    tq  = t * K                      (K = 1e9, Copy w/ scale)
  DVE    cmx = exclusive-cummax(tq)       (tensor_tensor_scan max, shifted AP)
  DVE    M   = min(p, t);  X = max(p, t)  (min/max are DVE-only here;
                                           [128, S+1] tiles, zero column 0)
  GPSIMD i0  = M[:,1:] - X[:,:-1]
  GPSIMD u0  = X[:,1:] - M[:,:-1]
  DVE    u'  = max(cmx - 0.95K, u0), accum -> ia    (invalid lanes -> V=0.05K;
                                                     ia = n_invalid*V + O(1e3))
  ACT    lnu = Ln(u');  r = Exp(-lnu) = 1/u'        (invalid -> 2e-8)
  DVE    junk= max(i0,0) * r, accum -> rowsum       (relu fused here)
Epilogue on [128, 8]: num_seg = S - round(ia / V) (2^23 magic rounding),
row_iou = rowsum / num_seg -> DMA out. Host: loss = 1 - sum(row_iou) / B.

Manual software pipelining: engine queues run in EMISSION order, so the
um pass (which waits on GPSIMD's u0) is emitted one tile late and the
Ln/Exp/final passes two tiles late. This hides the Pool and ACT latency
behind the next tile's DVE work: sim went 140 us -> 104 us per core.

The masked lanes contribute |inter|/V <= 2048 * 2e-8 ~ 4e-5 absolute to a
rowsum of O(1..30): negligible. num_seg recovery is exact (error << 0.5).
"""

import numpy as np

B, S = 8192, 2048
NCORES = 8
ROWS_PER_CORE = B // NCORES  # 1024
TILES = ROWS_PER_CORE // 128  # 8

K_SCALE = np.float32(1.0e9)  # ACT Ln accurate to ~1e16; keep u' moderate
C_THRESH = np.float32(0.95) * K_SCALE
V_INVALID = float(np.float32(K_SCALE - np.float32(C_THRESH)))  # invalid-lane u'
MAGIC = 8388608.0  # 2**23: float add/sub rounds to nearest integer

C_SPLIT = 1664  # DVE computes max on cols [0,C_SPLIT); Pool derives the rest
                # as (p+t) - min  (1-ulp difference, numerically validated)

UM_SKEW = 1  # um pass trails stage A by one tile
B_SKEW = 2  # ln/exp/final trail stage A by two tiles

_NC_CACHE = None

_RANGE_CLEAR_OPCODE = 176  # EVENT_SEMAPHORE_RANGE_CLEAR


def _legalize_waits(nc, maxw=1):
    """Make the Tile-generated module compatible with this walrus build.

    1. Drop tail EVENT_SEMAPHORE_RANGE_CLEAR InstISA ops (NRT re-initializes
       semaphore state per execution; this walrus rejects the encoding).
    2. Split instructions carrying more than `maxw` sync waits: excess waits
       move to carrier EventSemaphore nops inserted just before, same engine.
    """
    import concourse.mybir as mybir

    uid = [0]
    for fn in nc.m.functions:
        for blk in fn.blocks:
            lst = blk.instructions
            k = 0
            while k < len(lst):
                inst = lst[k]
                if (
                    type(inst).__name__ == "InstISA"
                    and getattr(inst, "isa_opcode", None) == _RANGE_CLEAR_OPCODE
                ):
                    si = inst.sync_info
                    if si is not None and (si.on_wait or si.on_update):
                        carrier = mybir.InstEventSemaphore(name=f"RCW-{uid[0]}")
                        uid[0] += 1
                        carrier.engine = inst.engine
                        carrier.sync_info = si
                        lst[k] = carrier
                        k += 1
                    else:
                        del lst[k]
                    continue
                si = inst.sync_info
                if si is not None and si.on_wait and len(si.on_wait) > maxw:
                    waits = list(si.on_wait)
                    extra, keep = waits[:-maxw], waits[-maxw:]
                    pos = k
                    for j in range(0, len(extra), maxw):
                        carrier = mybir.InstEventSemaphore(name=f"EVW-{uid[0]}")
                        uid[0] += 1
                        carrier.engine = inst.engine
                        carrier.sync_info = mybir.SyncInfo(
                            on_wait=extra[j : j + maxw], on_update=[]
                        )
                        lst.insert(pos, carrier)
                        pos += 1
                        k += 1
                    inst.sync_info = mybir.SyncInfo(
                        on_wait=keep, on_update=list(si.on_update)
                    )
                k += 1
    return nc


def _build_nc():
    import concourse.bass as bass
    import concourse.mybir as mybir
    from concourse.tile import TileContext

    f32 = mybir.dt.float32
    alu = mybir.AluOpType
    act = mybir.ActivationFunctionType

    nc = bass.Bass()
    p_d = nc.dram_tensor("predictions", [ROWS_PER_CORE, S], f32, kind="ExternalInput")
    t_d = nc.dram_tensor("targets", [ROWS_PER_CORE, S], f32, kind="ExternalInput")
    o_d = nc.dram_tensor("row_iou", [128, TILES], f32, kind="ExternalOutput")

    with TileContext(nc) as tc:
        with (
            tc.tile_pool(name="io", bufs=2) as iop,
            tc.tile_pool(name="geom", bufs=2) as gp,
            tc.tile_pool(name="cmxp", bufs=3) as cp,
            tc.tile_pool(name="i0p", bufs=2) as i0p,
            tc.tile_pool(name="u0p", bufs=2) as u0p,
            tc.tile_pool(name="ump", bufs=3) as ump,
            tc.tile_pool(name="uch", bufs=2) as up,
            tc.tile_pool(name="sp", bufs=2) as spp,
            tc.tile_pool(name="smp", bufs=1) as smp,
        ):
            acc_sb = smp.tile([128, TILES], f32, tag="acc")
            rs_sb = smp.tile([128, TILES], f32, tag="rs")
            carr = smp.tile([128, TILES], f32, tag="carr")
            nc.vector.memset(carr[:], float(S) + MAGIC)
            w_sb = smp.tile([128, TILES], f32, tag="w")
            w2_sb = smp.tile([128, TILES], f32, tag="w2")

            st_a = {}
            st_u = {}

            def stage_a(i):
                rows = slice(i * 128, (i + 1) * 128)
                p = iop.tile([128, S], f32, tag="p")
                t = iop.tile([128, S], f32, tag="t")
                tq = gp.tile([128, S], f32, tag="tq")
                cmx = cp.tile([128, S], f32, tag="cmx")
                M = gp.tile([128, S + 1], f32, tag="M")
                X = gp.tile([128, S + 1], f32, tag="X")
                sv = spp.tile([128, S - C_SPLIT], f32, tag="s")
                nc.scalar.memzero(cmx[:, 0:1])
                nc.scalar.memzero(M[:, 0:1])
                nc.gpsimd.memset(X[:, 0:1], 0.0)
                # tile 0 is processed in column quarters so DVE ramps up while
                # the rest of the data is still in flight (chained scan).
                nparts = 4 if i == 0 else 1
                Hc = S // nparts
                for k in range(nparts):
                    a, b = k * Hc, (k + 1) * Hc
                    nc.sync.dma_start(out=t[:, a:b], in_=t_d[rows, a:b])
                    nc.sync.dma_start(out=p[:, a:b], in_=p_d[rows, a:b])
                    nc.scalar.activation(
                        out=tq[:, a:b], in_=t[:, a:b], func=act.Copy,
                        scale=float(K_SCALE),
                    )
                    # min first: Pool's derived-max part waits on M
                    nc.vector.tensor_tensor(
                        out=M[:, a + 1 : b + 1], in0=p[:, a:b], in1=t[:, a:b],
                        op=alu.min,
                    )
                    last = k == nparts - 1
                    nc.vector.tensor_tensor_scan(
                        out=cmx[:, a + 1 : (b if last else b + 1)],
                        data0=tq[:, a : (b - 1 if last else b)],
                        data1=tq[:, a : (b - 1 if last else b)],
                        initial=(0.0 if k == 0 else cmx[:, a : a + 1]),
                        op0=alu.max,
                        op1=alu.bypass,
                    )
                    hi = min(b, C_SPLIT)
                    if hi > a:
                        nc.vector.tensor_tensor(
                            out=X[:, a + 1 : hi + 1], in0=p[:, a:hi],
                            in1=t[:, a:hi], op=alu.max,
                        )
                # Pool derives the remaining max columns: X = (p+t) - M
                nc.gpsimd.tensor_tensor(
                    out=sv[:], in0=p[:, C_SPLIT:S], in1=t[:, C_SPLIT:S],
                    op=alu.add,
                )
                nc.gpsimd.tensor_tensor(
                    out=X[:, C_SPLIT + 1 : S + 1], in0=sv[:],
                    in1=M[:, C_SPLIT + 1 : S + 1], op=alu.subtract,
                )
                i0 = i0p.tile([128, S], f32, tag="i0")
                nc.gpsimd.tensor_tensor(
                    out=i0[:], in0=M[:, 1 : S + 1], in1=X[:, 0:S], op=alu.subtract
                )
                u0 = u0p.tile([128, S], f32, tag="u0")
                nc.gpsimd.tensor_tensor(
                    out=u0[:], in0=X[:, 1 : S + 1], in1=M[:, 0:S], op=alu.subtract
                )
                st_a[i] = (i0, u0, cmx)

            def stage_u(i, split=False):
                i0, u0, cmx = st_a.pop(i)
                um = ump.tile([128, S], f32, tag="um")
                if split:
                    # last tile: halve the mask pass so Ln can start sooner;
                    # the two partial accums add up to the same ia.
                    Hh = S // 2
                    nc.vector.scalar_tensor_tensor(
                        out=um[:, 0:Hh], in0=cmx[:, 0:Hh],
                        scalar=float(C_THRESH), in1=u0[:, 0:Hh],
                        op0=alu.subtract, op1=alu.max,
                        accum_out=w2_sb[:, i : i + 1],
                    )
                    nc.vector.scalar_tensor_tensor(
                        out=um[:, Hh:S], in0=cmx[:, Hh:S],
                        scalar=float(C_THRESH), in1=u0[:, Hh:S],
                        op0=alu.subtract, op1=alu.max,
                        accum_out=acc_sb[:, i : i + 1],
                    )
                    nc.vector.tensor_tensor(
                        out=acc_sb[:, i : i + 1], in0=acc_sb[:, i : i + 1],
                        in1=w2_sb[:, i : i + 1], op=alu.add,
                    )
                else:
                    nc.vector.scalar_tensor_tensor(
                        out=um[:],
                        in0=cmx[:],
                        scalar=float(C_THRESH),
                        in1=u0[:],
                        op0=alu.subtract,
                        op1=alu.max,
                        accum_out=acc_sb[:, i : i + 1],
                    )
                st_u[i] = (i0, um)

            def stage_b(i, split=False):
                i0, um = st_u.pop(i)
                lnu = up.tile([128, S], f32, tag="lnu")
                r = up.tile([128, S], f32, tag="r")
                if split:
                    # last tile: halve the Ln/Exp/final chain to shrink the
                    # serial drain tail; partial row-sums add up afterwards.
                    Hh = S // 2
                    nc.scalar.activation(out=lnu[:, 0:Hh], in_=um[:, 0:Hh],
                                         func=act.Ln)
                    nc.scalar.activation(out=r[:, 0:Hh], in_=lnu[:, 0:Hh],
                                         func=act.Exp, scale=-1.0)
                    nc.scalar.activation(out=lnu[:, Hh:S], in_=um[:, Hh:S],
                                         func=act.Ln)
                    nc.scalar.activation(out=r[:, Hh:S], in_=lnu[:, Hh:S],
                                         func=act.Exp, scale=-1.0)
                    junk = ump.tile([128, S], f32, tag="um")
                    nc.vector.scalar_tensor_tensor(
                        out=junk[:, 0:Hh], in0=i0[:, 0:Hh], scalar=0.0,
                        in1=r[:, 0:Hh], op0=alu.max, op1=alu.mult,
                        accum_out=w_sb[:, i : i + 1],
                    )
                    nc.vector.scalar_tensor_tensor(
                        out=junk[:, Hh:S], in0=i0[:, Hh:S], scalar=0.0,
                        in1=r[:, Hh:S], op0=alu.max, op1=alu.mult,
                        accum_out=rs_sb[:, i : i + 1],
                    )
                    nc.vector.tensor_tensor(
                        out=rs_sb[:, i : i + 1], in0=rs_sb[:, i : i + 1],
                        in1=w_sb[:, i : i + 1], op=alu.add,
                    )
                else:
                    nc.scalar.activation(out=lnu[:], in_=um[:], func=act.Ln)
                    nc.scalar.activation(out=r[:], in_=lnu[:], func=act.Exp,
                                         scale=-1.0)
                    nc.vector.scalar_tensor_tensor(
                        out=um[:],
                        in0=i0[:],
                        scalar=0.0,
                        in1=r[:],
                        op0=alu.max,
                        op1=alu.mult,
                        accum_out=rs_sb[:, i : i + 1],
                    )

            for i in range(TILES):
                stage_a(i)
                if i >= UM_SKEW:
                    stage_u(i - UM_SKEW)
                if i >= B_SKEW:
                    stage_b(i - B_SKEW)
            for i in range(TILES - UM_SKEW, TILES):
                stage_u(i, split=(i == TILES - 1))
            for i in range(TILES - B_SKEW, TILES):
                stage_b(i, split=(i == TILES - 1))

            # epilogue: num_seg = (S + MAGIC - acc/V) - MAGIC; out = rs/num_seg
            nc.vector.scalar_tensor_tensor(
                out=w_sb[:], in0=acc_sb[:], scalar=-1.0 / V_INVALID, in1=carr[:],
                op0=alu.mult, op1=alu.add,
            )
            nc.vector.tensor_scalar(
                out=carr[:], in0=w_sb[:], scalar1=MAGIC, scalar2=None,
                op0=alu.subtract,
            )
            nc.vector.reciprocal(out=w_sb[:], in_=carr[:])
            nc.vector.tensor_tensor(
                out=carr[:], in0=rs_sb[:], in1=w_sb[:], op=alu.mult
            )
            nc.sync.dma_start(out=o_d[:, :], in_=carr[:])
    return _legalize_waits(nc)


def kernel(predictions: np.ndarray, targets: np.ndarray) -> np.ndarray:
    global _NC_CACHE
    from concourse.bass_utils import run_bass_kernel_spmd

    if _NC_CACHE is None:
        _NC_CACHE = _build_nc()
    nc = _NC_CACHE

    p = np.ascontiguousarray(predictions, dtype=np.float32)
    t = np.ascontiguousarray(targets, dtype=np.float32)
    in_maps = []
    for c in range(NCORES):
        sl = slice(c * ROWS_PER_CORE, (c + 1) * ROWS_PER_CORE)
        in_maps.append({"predictions": p[sl], "targets": t[sl]})
    res = run_bass_kernel_spmd(nc, in_maps, core_ids=list(range(NCORES)))
    total = 0.0
    for rmap in res.results:
        total += float(rmap["row_iou"].astype(np.float64).sum())
    return np.asarray(1.0 - total / B, dtype=np.float32)
